# revision 2
# baseline (speedup 1.0000x reference)
"""MLA (CustomLlamaMLAForInfer) Trainium2 Bass kernel.

Sharding: tensor-parallel over heads across 8 NeuronCores. Core c owns
kv-head c and q-heads [4c, 4c+4). Every core sees the full token stream
(B*S = 4096 tokens); o_proj is computed against the core's 512
head-dims, producing a partial [4096, 4096] output that the host sums
across the 8 cores.

Device program phases (single SPMD program, per-core weights differ):
  1a. qT = Wq_shard @ hidden.T   (rope + 1/sqrt(d) folded in at evict)
  1b. c_kvT = Wdk @ hidden.T ; krT = Wkr_shard @ hidden.T (rope at evict)
  2.  k_c / v from c_kvT via Wupk/Wupv shards; assemble kT_full, v_tok
  3.  causal attention per (batch, q-head): scores_T = kT.T@qT blocks,
      exp (no max-sub needed: |scores| < ~6), mask diag blocks,
      out_T[d,q] += v_tok.T @ p_T, sums via ones-matmul, normalize
  4.  partial o_proj: out[tok, hid] += attn_T.T @ WoT_shard

All matmuls run as float32r (fp22 mantissa, 1 PE pass).
"""

import numpy as np

HIDDEN = 4096
N_HEADS = 32
KV_HEADS = 8
HEAD_DIM = 128
LOW_RANK = 64
TOP_K_ROPE = 32
ROPE_THETA = 10000.0
B, S = 2, 2048
NCORES = 8
HPC = N_HEADS // NCORES          # q heads per core = 4
QR = HPC * HEAD_DIM              # q rows per core = 512
CD = LOW_RANK * KV_HEADS         # latent dim = 512
KRR = 2 * TOP_K_ROPE             # rope rows per kv head = 64


def _rope_tables(seq_len):
    inv = 1.0 / (ROPE_THETA ** (np.arange(0, HEAD_DIM, 2, dtype=np.float32) / HEAD_DIM))
    pos = np.arange(seq_len, dtype=np.float32)
    fr = np.outer(pos, inv)
    emb = np.concatenate([fr, fr], axis=-1)          # [S, 128]
    return (np.cos(emb).T.astype(np.float32),        # [128, S]
            np.sin(emb).T.astype(np.float32))


def build_program(Bv=B, Sv=S, TB=512, QB=512, trace_sim=False):
    """Build the SPMD Bass program. TB = proj token-block, QB = attention
    q-block (both <= 512, the fp32 moving-operand limit)."""
    from concourse import bacc, tile, mybir
    import concourse.bass as bass

    f32 = mybir.dt.float32
    F32R = mybir.dt.float32r
    MS = bass.MemorySpace
    EXP = mybir.ActivationFunctionType.Exp

    NT = Bv * Sv                 # total tokens
    HT = HIDDEN // 128           # hidden tiles = 32
    NTB = NT // TB               # proj token blocks
    NQB = Sv // QB               # q blocks per batch
    NJ = QB // 128               # diagonal mask variants
    NKT_B = Sv // 128            # k tiles per batch
    QT = QR // 128               # q-head tiles per core = 4
    LT = CD // 128               # latent tiles = 4

    nc = bacc.Bacc("TRN2", target_bir_lowering=False, debug=False,
                   num_devices=NCORES)

    def din(name, shape):
        return nc.dram_tensor(name, shape, f32, kind="ExternalInput").ap()

    hidT = din("hidT", [HIDDEN, NT])
    wq = din("wq_t", [HIDDEN, QR])
    wkd = din("wkd_t", [HIDDEN, CD + KRR])
    wupk = din("wupk_t", [CD, KRR])
    wupv = din("wupv_t", [CD, HEAD_DIM])
    wo = din("wo_t", [QR, HIDDEN])
    qcos = din("qcos", [128, NT])
    qsin = din("qsin", [128, NT])
    kcos = din("kcos", [KRR, NT])
    ksin = din("ksin", [KRR, NT])
    masks = din("masks", [128, NJ, QB])
    onesd = din("ones", [128, 1])
    outp = nc.dram_tensor("out_part", [NT, HIDDEN], f32, kind="ExternalOutput").ap()
    qT_s = nc.dram_tensor("qT_s", [QT, 128, NT], f32).ap()
    ckv_s = nc.dram_tensor("ckv_s", [LT, 128, NT], f32).ap()

    with tile.TileContext(nc, trace_sim=trace_sim) as tc:
        with tc.tile_pool(name="persist", bufs=1) as pers:
            kT_full = pers.tile([128, NT], F32R, tag="kT")
            v_tok = pers.tile([128, NT // 128, HEAD_DIM], F32R, tag="vtok")

            # ---------------- phase 1: projections of hidden ----------------
            with tc.tile_pool(name="cos", bufs=1) as cp:
                qcos_sb = cp.tile([128, NT], f32, tag="qc")
                qsin_sb = cp.tile([128, NT], f32, tag="qs")
                kcos_sb = cp.tile([KRR, NT], f32, tag="kc")
                ksin_sb = cp.tile([KRR, NT], f32, tag="ks")
                nc.sync.dma_start(qcos_sb[:], qcos)
                nc.sync.dma_start(qsin_sb[:], qsin)
                nc.sync.dma_start(kcos_sb[:], kcos)
                nc.sync.dma_start(ksin_sb[:], ksin)

                # ---- pass A: q projection (+rope, +1/sqrt(d) via tables) ----
                with tc.tile_pool(name="wqp", bufs=1) as wqp, \
                     tc.tile_pool(name="hidA", bufs=8) as hpA, \
                     tc.tile_pool(name="stA", bufs=2) as stA, \
                     tc.tile_pool(name="psA", bufs=8, space=MS.PSUM) as ppA:
                    wq_sb = wqp.tile([128, HT, QR], F32R)
                    nc.sync.dma_start(wq_sb[:], wq.rearrange("(t p) w -> p t w", p=128).bitcast(F32R))
                    for blk in range(NTB):
                        c0, c1 = blk * TB, (blk + 1) * TB
                        qps = [ppA.tile([128, TB], f32, tag="qps", name=f"qps{_m}") for _m in range(QT)]
                        for t in range(HT):
                            ht = hpA.tile([128, TB], F32R, tag="hid")
                            nc.sync.dma_start(ht[:], hidT[t * 128:(t + 1) * 128, c0:c1].bitcast(F32R))
                            for m in range(QT):
                                nc.tensor.matmul(
                                    qps[m][:],
                                    wq_sb[:, t, m * 128:(m + 1) * 128],
                                    ht[:],
                                    start=(t == 0), stop=(t == HT - 1))
                        for m in range(QT):
                            raw = stA.tile([128, TB], f32, tag="raw")
                            nc.scalar.copy(raw[:], qps[m][:])
                            rot = stA.tile([128, TB], f32, tag="rot")
                            nc.sync.dma_start(rot[0:64, :], raw[64:128, :])
                            nc.sync.dma_start(rot[64:128, :], raw[0:64, :])
                            qsb = stA.tile([128, TB], f32, tag="qsb")
                            nc.vector.tensor_mul(qsb[:], raw[:], qcos_sb[:, c0:c1])
                            nc.vector.tensor_mul(rot[:], rot[:], qsin_sb[:, c0:c1])
                            nc.vector.tensor_add(qsb[:], qsb[:], rot[:])
                            nc.sync.dma_start(qT_s[m, :, c0:c1], qsb[:])

                # ---- pass B: c_kv (latent) + k_rope projections ----
                with tc.tile_pool(name="wkdp", bufs=1) as wkdp, \
                     tc.tile_pool(name="hidB", bufs=8) as hpB, \
                     tc.tile_pool(name="stB", bufs=2) as stB, \
                     tc.tile_pool(name="psB", bufs=6, space=MS.PSUM) as ppB, \
                     tc.tile_pool(name="psBk", bufs=2, space=MS.PSUM) as ppBk:
                    wkd_sb = wkdp.tile([128, HT, CD + KRR], F32R)
                    nc.sync.dma_start(wkd_sb[:], wkd.rearrange("(t p) w -> p t w", p=128).bitcast(F32R))
                    for blk in range(NTB):
                        c0, c1 = blk * TB, (blk + 1) * TB
                        dps = [ppB.tile([128, TB], f32, tag="dps", name=f"dps{_m}") for _m in range(LT)]
                        krp = ppBk.tile([KRR, TB], f32, tag="krp")
                        for t in range(HT):
                            ht = hpB.tile([128, TB], F32R, tag="hid")
                            nc.sync.dma_start(ht[:], hidT[t * 128:(t + 1) * 128, c0:c1].bitcast(F32R))
                            for m in range(LT):
                                nc.tensor.matmul(
                                    dps[m][:],
                                    wkd_sb[:, t, m * 128:(m + 1) * 128],
                                    ht[:],
                                    start=(t == 0), stop=(t == HT - 1))
                            nc.tensor.matmul(
                                krp[:],
                                wkd_sb[:, t, CD:CD + KRR],
                                ht[:],
                                start=(t == 0), stop=(t == HT - 1))
                        for m in range(LT):
                            csb = stB.tile([128, TB], f32, tag="csb")
                            nc.scalar.copy(csb[:], dps[m][:])
                            nc.sync.dma_start(ckv_s[m, :, c0:c1], csb[:])
                        # rope the 64 k-rope rows, scatter into kT_full
                        rawk = stB.tile([KRR, TB], f32, tag="rawk")
                        nc.scalar.copy(rawk[:], krp[:])
                        rotk = stB.tile([KRR, TB], f32, tag="rotk")
                        nc.sync.dma_start(rotk[0:32, :], rawk[32:64, :])
                        nc.sync.dma_start(rotk[32:64, :], rawk[0:32, :])
                        ksb = stB.tile([KRR, TB], f32, tag="ksb")
                        nc.vector.tensor_mul(ksb[:], rawk[:], kcos_sb[:, c0:c1])
                        nc.vector.tensor_mul(rotk[:], rotk[:], ksin_sb[:, c0:c1])
                        nc.vector.tensor_add(ksb[:], ksb[:], rotk[:])
                        nc.sync.dma_start(kT_full[0:32, c0:c1], ksb[0:32, :].bitcast(F32R))
                        nc.sync.dma_start(kT_full[64:96, c0:c1], ksb[32:64, :].bitcast(F32R))

            # ---------------- phase 2: k_c and v from the latent ----------------
            with tc.tile_pool(name="wup", bufs=1) as wup, \
                 tc.tile_pool(name="ckvb", bufs=2) as ckvb, \
                 tc.tile_pool(name="st2", bufs=2) as st2, \
                 tc.tile_pool(name="psK", bufs=2, space=MS.PSUM) as psK, \
                 tc.tile_pool(name="psV", bufs=4, space=MS.PSUM) as psV:
                wupk_sb = wup.tile([128, LT, KRR], F32R, tag="upk")
                wupv_sb = wup.tile([128, LT, HEAD_DIM], F32R, tag="upv")
                nc.sync.dma_start(wupk_sb[:], wupk.rearrange("(t p) w -> p t w", p=128).bitcast(F32R))
                nc.sync.dma_start(wupv_sb[:], wupv.rearrange("(t p) w -> p t w", p=128).bitcast(F32R))
                for blk in range(NTB):
                    c0, c1 = blk * TB, (blk + 1) * TB
                    cb = ckvb.tile([128, LT, TB], F32R, tag="cb")
                    nc.sync.dma_start(cb[:], ckv_s[:, :, c0:c1].rearrange("t p w -> p t w").bitcast(F32R))
                    kcp = psK.tile([KRR, TB], f32, tag="kcp")
                    for lt in range(LT):
                        nc.tensor.matmul(kcp[:],
                                         wupk_sb[:, lt, :],
                                         cb[:, lt, :],
                                         start=(lt == 0), stop=(lt == LT - 1))
                    kcs = st2.tile([KRR, TB], f32, tag="kcs")
                    nc.scalar.copy(kcs[:], kcp[:])
                    nc.sync.dma_start(kT_full[32:64, c0:c1], kcs[0:32, :].bitcast(F32R))
                    nc.sync.dma_start(kT_full[96:128, c0:c1], kcs[32:64, :].bitcast(F32R))
                    for tt in range(TB // 128):
                        vp = psV.tile([128, HEAD_DIM], f32, tag="vp")
                        for lt in range(LT):
                            nc.tensor.matmul(
                                vp[:],
                                cb[:, lt, tt * 128:(tt + 1) * 128],
                                wupv_sb[:, lt, :],
                                start=(lt == 0), stop=(lt == LT - 1))
                        nc.scalar.copy(v_tok[:, blk * (TB // 128) + tt, :], vp[:])

            # ---------------- phases 3+4 ----------------
            with tc.tile_pool(name="attn", bufs=1) as ap_:
                attn_sb = ap_.tile([128, QT, NT], F32R)

                with tc.tile_pool(name="qh", bufs=2) as qhp, \
                     tc.tile_pool(name="cst3", bufs=1) as cst3, \
                     tc.tile_pool(name="pt", bufs=3) as ptp, \
                     tc.tile_pool(name="sm", bufs=2) as smp, \
                     tc.tile_pool(name="psS", bufs=3, space=MS.PSUM) as psS, \
                     tc.tile_pool(name="psO", bufs=2, space=MS.PSUM) as psO, \
                     tc.tile_pool(name="psU", bufs=2, space=MS.PSUM) as psU:
                    masks_sb = cst3.tile([128, NJ, QB], F32R, tag="masks")
                    nc.sync.dma_start(masks_sb[:], masks.bitcast(F32R))
                    ones_sb = cst3.tile([128, 1], F32R, tag="ones")
                    nc.sync.dma_start(ones_sb[:], onesd.bitcast(F32R))
                    for h in range(QT):
                        qh_sb = qhp.tile([128, NT], F32R, tag="qh")
                        nc.sync.dma_start(qh_sb[:], qT_s[h].bitcast(F32R))
                        for b in range(Bv):
                            off = b * Sv
                            for qb in range(NQB):
                                ops = psO.tile([128, QB], f32, tag="ops")
                                sps = psU.tile([1, QB], f32, tag="sps")
                                nkt = (qb + 1) * NJ
                                for kt in range(nkt):
                                    scp = psS.tile([128, QB], f32, tag="scp")
                                    nc.tensor.matmul(
                                        scp[:],
                                        kT_full[:, off + kt * 128: off + (kt + 1) * 128],
                                        qh_sb[:, off + qb * QB: off + (qb + 1) * QB],
                                        start=True, stop=True)
                                    ptile = ptp.tile([128, QB], F32R, tag="pt")
                                    nc.scalar.activation(ptile[:], scp[:], EXP)
                                    j = kt - qb * NJ
                                    if j >= 0:
                                        nc.vector.tensor_mul(ptile[:], ptile[:], masks_sb[:, j, :])
                                    nc.tensor.matmul(
                                        ops[:],
                                        v_tok[:, b * NKT_B + kt, :],
                                        ptile[:],
                                        start=(kt == 0), stop=(kt == nkt - 1))
                                    nc.tensor.matmul(
                                        sps[:],
                                        ones_sb[:],
                                        ptile[:],
                                        start=(kt == 0), stop=(kt == nkt - 1))
                                rec = smp.tile([1, QB], f32, tag="rec")
                                nc.vector.reciprocal(rec[:], sps[:])
                                rb = smp.tile([128, QB], f32, tag="rb")
                                nc.gpsimd.partition_broadcast(rb[:], rec[:])
                                nc.vector.tensor_mul(
                                    attn_sb[:, h, off + qb * QB: off + (qb + 1) * QB],
                                    ops[:], rb[:])

                # ---- phase 4: partial o_proj ----
                with tc.tile_pool(name="wop", bufs=1) as wop, \
                     tc.tile_pool(name="st4", bufs=4) as st4, \
                     tc.tile_pool(name="ps4", bufs=6, space=MS.PSUM) as ps4:
                    wo_sb = wop.tile([128, QT, HIDDEN], F32R)
                    nc.sync.dma_start(wo_sb[:], wo.rearrange("(t p) w -> p t w", p=128).bitcast(F32R))
                    for T in range(NT // 128):
                        for n in range(HIDDEN // 512):
                            ps = ps4.tile([128, 512], f32, tag="ps")
                            for h2 in range(QT):
                                nc.tensor.matmul(
                                    ps[:],
                                    attn_sb[:, h2, T * 128:(T + 1) * 128],
                                    wo_sb[:, h2, n * 512:(n + 1) * 512],
                                    start=(h2 == 0), stop=(h2 == QT - 1))
                            osb = st4.tile([128, 512], f32, tag="osb")
                            nc.vector.tensor_copy(osb[:], ps[:])
                            nc.sync.dma_start(outp[T * 128:(T + 1) * 128, n * 512:(n + 1) * 512], osb[:])

    nc.compile()
    return nc


def make_in_maps(hidden_states, Wq, Wkr, Wdk, Wupk, Wupv, Wo, Bv=B, Sv=S, QB=512):
    """Host-side sharding + layout prep. Returns per-core input dicts."""
    NT = Bv * Sv
    NJ = QB // 128
    scale = 1.0 / np.sqrt(np.float32(HEAD_DIM))

    hidT = np.ascontiguousarray(
        hidden_states.reshape(NT, HIDDEN).T.astype(np.float32))

    cos_t, sin_t = _rope_tables(Sv)                    # [128, S]
    cos_t = np.tile(cos_t, (1, Bv))                    # [128, NT]
    sin_t = np.tile(sin_t, (1, Bv))
    qcos = np.ascontiguousarray(cos_t * scale)
    qsin = np.ascontiguousarray(
        np.concatenate([-sin_t[0:64], sin_t[64:128]], axis=0) * scale)
    kcos = np.ascontiguousarray(
        np.concatenate([cos_t[0:32], cos_t[64:96]], axis=0))
    ksin = np.ascontiguousarray(
        np.concatenate([-sin_t[0:32], sin_t[64:96]], axis=0))

    k_idx = np.arange(128)[:, None]
    q_idx = np.arange(QB)[None, :]
    masks = np.stack(
        [(q_idx >= j * 128 + k_idx).astype(np.float32) for j in range(NJ)],
        axis=1)                                        # [128, NJ, QB]
    masks = np.ascontiguousarray(masks)

    in_maps = []
    for c in range(NCORES):
        wq_t = np.ascontiguousarray(Wq[QR * c:QR * (c + 1)].T.astype(np.float32))
        wkd_t = np.ascontiguousarray(
            np.concatenate([Wdk, Wkr[KRR * c:KRR * (c + 1)]], axis=0).T.astype(np.float32))
        wupk_t = np.ascontiguousarray(Wupk[KRR * c:KRR * (c + 1)].T.astype(np.float32))
        wupv_t = np.ascontiguousarray(
            Wupv[HEAD_DIM * c:HEAD_DIM * (c + 1)].T.astype(np.float32))
        wo_t = np.ascontiguousarray(Wo[:, QR * c:QR * (c + 1)].T.astype(np.float32))
        in_maps.append({
            "hidT": hidT, "wq_t": wq_t, "wkd_t": wkd_t,
            "wupk_t": wupk_t, "wupv_t": wupv_t, "wo_t": wo_t,
            "qcos": qcos, "qsin": qsin, "kcos": kcos, "ksin": ksin,
            "masks": masks, "ones": np.ones((128, 1), np.float32),
        })
    return in_maps


_NC_CACHE = {}


def _get_program(key=(B, S, 512, 512)):
    if key not in _NC_CACHE:
        _NC_CACHE[key] = build_program(*key)
    return _NC_CACHE[key]


def finish_output(res):
    out = res.results[0]["out_part"].astype(np.float32)
    for i in range(1, NCORES):
        out = out + res.results[i]["out_part"]
    return out.reshape(B, S, HIDDEN).astype(np.float32)


def kernel(hidden_states, Wq, Wkr, Wdk, Wupk, Wupv, Wo):
    from concourse.bass_utils import run_bass_kernel_spmd

    hidden_states = np.asarray(hidden_states)
    in_maps = make_in_maps(hidden_states, np.asarray(Wq), np.asarray(Wkr),
                           np.asarray(Wdk), np.asarray(Wupk), np.asarray(Wupv),
                           np.asarray(Wo))
    nc = _get_program()
    res = run_bass_kernel_spmd(nc, in_maps, list(range(NCORES)))
    return finish_output(res)



# revision 3
# speedup vs baseline: 1.1148x; 1.1148x over previous
"""MLA (CustomLlamaMLAForInfer) Trainium2 Bass kernel, v2.

Sharding: tensor-parallel over heads across 8 NeuronCores. Core c owns
kv-head c and q-heads [4c, 4c+4). Every core sees the full token stream
(B*S = 4096 tokens). The shared low-rank latent (c_kv, 512 dims) is
*sharded*: core c computes latent dims [64c, 64c+64) for all tokens and
an AllGather rebuilds the full latent on every core. o_proj is
row-sharded; the host sums the 8 partial [4096, 4096] outputs.

All compute in bf16 (inputs pre-converted host-side), fp32 PSUM
accumulation. Device program (single SPMD NEFF, per-core weights
differ), one streaming TileContext:

  A(g), g=0..7 (512-token blocks): qT = Wq_shard @ hid.T (rope folded
     in at evict, kept in SBUF), [c_kv shard; k_rope shard] = one
     fused matmul; c_kv shard -> DRAM -> AllGather (overlapped with
     A(g+1..)); k_rope roped+scattered into SBUF kT.
  B(g): k_nope/v of the core's kv head from the gathered latent.
  C(g): causal attention for q-block g, 4 q-heads; scores_T = kT.T@qT
     per 128-k-tile, exp on ScalarE (no max-sub: |scores| < ~6),
     diag-block masks, out_T[d,q] += v.T @ p, denom via ones-matmul.
  D(g): partial o_proj for block g's 512 tokens -> bf16 out_part.

PSUM budget (8 banks): qps ring 4 (A q-heads / B v / D o_proj),
sc ring 2 (A dk+kr / B k_nope / C scores), av 1, sum 1.
"""

import numpy as np

HIDDEN = 4096
N_HEADS = 32
KV_HEADS = 8
HEAD_DIM = 128
LOW_RANK = 64
TOP_K_ROPE = 32
ROPE_THETA = 10000.0
B, S = 2, 2048
NCORES = 8
HPC = N_HEADS // NCORES          # q heads per core = 4
QR = HPC * HEAD_DIM              # q rows per core = 512
CD = LOW_RANK * KV_HEADS         # latent dim = 512
LSH = CD // NCORES               # latent shard per core = 64
KRR = 2 * TOP_K_ROPE             # rope rows per kv head = 64
NT = B * S                       # total tokens = 4096
TB = 512                         # token block
NG = NT // TB                    # token blocks = 8
HT = HIDDEN // 128               # hidden k-tiles = 32
LT = CD // 128                   # latent k-tiles = 4
NJ = TB // 128                   # diag mask variants = 4


def _rope_tables(seq_len):
    inv = 1.0 / (ROPE_THETA ** (np.arange(0, HEAD_DIM, 2, dtype=np.float32) / HEAD_DIM))
    pos = np.arange(seq_len, dtype=np.float32)
    fr = np.outer(pos, inv)
    emb = np.concatenate([fr, fr], axis=-1)          # [S, 128]
    return (np.cos(emb).T.astype(np.float32),        # [128, S]
            np.sin(emb).T.astype(np.float32))


def build_program(trace_sim=False):
    from concourse import bacc, tile, mybir
    import concourse.bass as bass

    f32 = mybir.dt.float32
    bf16 = mybir.dt.bfloat16
    MS = bass.MemorySpace
    EXP = mybir.ActivationFunctionType.Exp

    nc = bacc.Bacc("TRN2", target_bir_lowering=False, debug=False,
                   num_devices=NCORES)

    def din(name, shape):
        return nc.dram_tensor(name, shape, bf16, kind="ExternalInput").ap()

    hidT = din("hidT", [HIDDEN, NT])
    wq = din("wq_t", [HIDDEN, QR])
    wdkkr = din("wdkkr_t", [HIDDEN, 128])
    wupk = din("wupk_t", [CD, KRR])
    wupv = din("wupv_t", [CD, HEAD_DIM])
    wo = din("wo_t", [QR, HIDDEN])
    qcos = din("qcos", [128, S])
    qsin = din("qsin", [128, S])
    kcos = din("kcos", [128, S])     # rows 64:128 hold the values
    ksin = din("ksin", [128, S])     # rows 64:128 hold the values
    masks = din("masks", [128, NJ, TB])
    outp = nc.dram_tensor("out_part", [NT, HIDDEN], bf16,
                          kind="ExternalOutput").ap()

    rg = [list(range(NCORES))]

    with tile.TileContext(nc, trace_sim=trace_sim) as tc:
        with tc.tile_pool(name="pers", bufs=1) as pers, \
             tc.tile_pool(name="ring", bufs=2) as ring, \
             tc.tile_pool(name="dram", bufs=8, space="DRAM") as dram, \
             tc.tile_pool(name="ps", bufs=1, space=MS.PSUM) as psp:

            # ---------------- persistent SBUF ----------------
            wq_sb = pers.tile([128, HT, QR], bf16, tag="wq")
            nc.sync.dma_start(wq_sb[:], wq.rearrange("(t p) w -> p t w", p=128))
            wdkkr_sb = pers.tile([128, HT, 128], bf16, tag="wdkkr")
            nc.sync.dma_start(wdkkr_sb[:], wdkkr.rearrange("(t p) w -> p t w", p=128))
            wupk_sb = pers.tile([128, LT, KRR], bf16, tag="wupk")
            nc.sync.dma_start(wupk_sb[:], wupk.rearrange("(t p) w -> p t w", p=128))
            wupv_sb = pers.tile([128, LT, HEAD_DIM], bf16, tag="wupv")
            nc.sync.dma_start(wupv_sb[:], wupv.rearrange("(t p) w -> p t w", p=128))
            wo_sb = pers.tile([128, HPC, HIDDEN], bf16, tag="wo")
            nc.sync.dma_start(wo_sb[:], wo.rearrange("(h p) w -> p h w", p=128))
            qcos_sb = pers.tile([128, S], bf16, tag="qcos")
            nc.sync.dma_start(qcos_sb[:], qcos)
            qsin_sb = pers.tile([128, S], bf16, tag="qsin")
            nc.sync.dma_start(qsin_sb[:], qsin)
            kcos_sb = pers.tile([128, S], bf16, tag="kcos")
            nc.sync.dma_start(kcos_sb[:], kcos)
            ksin_sb = pers.tile([128, S], bf16, tag="ksin")
            nc.sync.dma_start(ksin_sb[:], ksin)
            masks_sb = pers.tile([128, NJ, TB], bf16, tag="masks")
            nc.sync.dma_start(masks_sb[:], masks)
            ones_sb = pers.tile([128, 1], bf16, tag="ones")
            nc.vector.memset(ones_sb[:], 1.0)

            qT_sb = pers.tile([128, HPC, NT], bf16, tag="qT")
            kT_sb = pers.tile([128, NT], bf16, tag="kT")
            v_sb = pers.tile([128, NT // 128, HEAD_DIM], bf16, tag="v")

            gaths = []

            # ================ phase A: projections ================
            for g in range(NG):
                c0, c1 = g * TB, (g + 1) * TB
                p0 = (g % (S // TB)) * TB          # table col (per batch)
                p1 = p0 + TB
                qps = [psp.tile([128, TB], f32, tag="qps", bufs=4,
                                name=f"qps{g}_{m}") for m in range(HPC)]
                dkp = psp.tile([128, TB], f32, tag="sc", bufs=2,
                               name=f"dkp{g}")
                for t in range(HT):
                    ht = ring.tile([128, TB], bf16, tag="hid", bufs=10,
                                   name=f"hid{g}_{t}")
                    nc.sync.dma_start(ht[:], hidT[t * 128:(t + 1) * 128, c0:c1])
                    for m in range(HPC):
                        nc.tensor.matmul(qps[m][:],
                                         wq_sb[:, t, m * 128:(m + 1) * 128],
                                         ht[:],
                                         start=(t == 0), stop=(t == HT - 1))
                    nc.tensor.matmul(dkp[:], wdkkr_sb[:, t, :], ht[:],
                                     start=(t == 0), stop=(t == HT - 1))
                # ---- evict q heads with rope ----
                for m in range(HPC):
                    raw = ring.tile([128, TB], bf16, tag="raw", name=f"raw{g}_{m}")
                    nc.scalar.copy(raw[:], qps[m][:])
                    rot = ring.tile([128, TB], bf16, tag="rot", name=f"rot{g}_{m}")
                    nc.sync.dma_start(rot[0:64, :], raw[64:128, :])
                    nc.sync.dma_start(rot[64:128, :], raw[0:64, :])
                    qsb = ring.tile([128, TB], bf16, tag="qsb", name=f"qsb{g}_{m}")
                    nc.vector.tensor_mul(qsb[:], raw[:], qcos_sb[:, p0:p1])
                    nc.vector.tensor_mul(rot[:], rot[:], qsin_sb[:, p0:p1])
                    nc.vector.tensor_add(qT_sb[:, m, c0:c1], qsb[:], rot[:])
                # ---- evict latent shard + k rope ----
                dka = ring.tile([128, TB], bf16, tag="dka", name=f"dka{g}")
                nc.scalar.copy(dka[:], dkp[:])
                ckv_my = dram.tile([LSH, TB], bf16, tag="ckv_my", name=f"ckvmy{g}")
                nc.sync.dma_start(ckv_my[:], dka[0:LSH, :])
                gath = dram.tile([CD, TB], bf16, tag="gath", addr_space="Shared",
                                 name=f"gath{g}")
                nc.gpsimd.collective_compute(
                    "AllGather", mybir.AluOpType.bypass, replica_groups=rg,
                    ins=[ckv_my.opt()], outs=[gath.opt()])
                gaths.append(gath)
                # rope rows live at partitions 64:128
                rotk = ring.tile([128, TB], bf16, tag="rotk", name=f"rotk{g}")
                nc.sync.dma_start(rotk[64:96, :], dka[96:128, :])
                nc.sync.dma_start(rotk[96:128, :], dka[64:96, :])
                ktm = ring.tile([128, TB], bf16, tag="ktm", name=f"ktm{g}")
                nc.vector.tensor_mul(ktm[64:128, :], dka[64:128, :],
                                     kcos_sb[64:128, p0:p1])
                nc.vector.tensor_mul(rotk[64:128, :], rotk[64:128, :],
                                     ksin_sb[64:128, p0:p1])
                nc.vector.tensor_add(ktm[64:128, :], ktm[64:128, :],
                                     rotk[64:128, :])
                nc.sync.dma_start(kT_sb[0:32, c0:c1], ktm[64:96, :])
                nc.sync.dma_start(kT_sb[64:96, c0:c1], ktm[96:128, :])

            # ================ phases B/C/D per block ================
            for g in range(NG):
                c0, c1 = g * TB, (g + 1) * TB
                b, qb = g // (S // TB), g % (S // TB)
                off = b * S
                # ---- B: k_nope + v from gathered latent ----
                cb = ring.tile([128, LT, TB], bf16, tag="cb", name=f"cb{g}")
                nc.sync.dma_start(cb[:], gaths[g].rearrange("(l p) w -> p l w", p=128))
                kcp = psp.tile([KRR, TB], f32, tag="sc", bufs=2, name=f"kcp{g}")
                for lt in range(LT):
                    nc.tensor.matmul(kcp[:], wupk_sb[:, lt, :], cb[:, lt, :],
                                     start=(lt == 0), stop=(lt == LT - 1))
                kcs = ring.tile([KRR, TB], bf16, tag="kcs", name=f"kcs{g}")
                nc.scalar.copy(kcs[:], kcp[:])
                nc.sync.dma_start(kT_sb[32:64, c0:c1], kcs[0:32, :])
                nc.sync.dma_start(kT_sb[96:128, c0:c1], kcs[32:64, :])
                for tt in range(TB // 128):
                    vp = psp.tile([128, HEAD_DIM], f32, tag="qps", bufs=4,
                                  name=f"vp{g}_{tt}")
                    for lt in range(LT):
                        nc.tensor.matmul(vp[:],
                                         cb[:, lt, tt * 128:(tt + 1) * 128],
                                         wupv_sb[:, lt, :],
                                         start=(lt == 0), stop=(lt == LT - 1))
                    nc.scalar.copy(v_sb[:, g * (TB // 128) + tt, :], vp[:])

                # ---- C: attention for q-block (b, qb), 4 heads ----
                atn = ring.tile([128, HPC, TB], bf16, tag="atn", name=f"atn{g}")
                nkt = (qb + 1) * NJ
                for h in range(HPC):
                    ops = psp.tile([128, TB], f32, tag="av", bufs=1,
                                   name=f"ops{g}_{h}")
                    sps = psp.tile([1, TB], f32, tag="sum", bufs=1,
                                   name=f"sps{g}_{h}")
                    ptiles = [None, None]
                    for kt in range(nkt + 1):
                        if kt < nkt:
                            scp = psp.tile([128, TB], f32, tag="sc", bufs=2,
                                           name=f"scp{g}_{h}_{kt}")
                            nc.tensor.matmul(
                                scp[:],
                                kT_sb[:, off + kt * 128: off + (kt + 1) * 128],
                                qT_sb[:, h, off + qb * TB: off + (qb + 1) * TB],
                                start=True, stop=True)
                            pt = ring.tile([128, TB], bf16, tag="pt", bufs=3,
                                           name=f"pt{g}_{h}_{kt}")
                            nc.scalar.activation(pt[:], scp[:], EXP)
                            j = kt - qb * NJ
                            if j >= 0:
                                nc.vector.tensor_mul(pt[:], pt[:], masks_sb[:, j, :])
                            ptiles[kt % 2] = pt
                        # one-stage software pipeline: AV/sum for kt-1
                        pk = kt - 1
                        if pk >= 0:
                            ppt = ptiles[pk % 2]
                            nc.tensor.matmul(ops[:],
                                             v_sb[:, b * (S // 128) + pk, :],
                                             ppt[:],
                                             start=(pk == 0), stop=(pk == nkt - 1))
                            nc.tensor.matmul(sps[:], ones_sb[:], ppt[:],
                                             start=(pk == 0), stop=(pk == nkt - 1))
                    rec = ring.tile([1, TB], f32, tag="rec", name=f"rec{g}_{h}")
                    nc.vector.reciprocal(rec[:], sps[:])
                    rb = ring.tile([128, TB], f32, tag="rb", name=f"rb{g}_{h}")
                    nc.gpsimd.partition_broadcast(rb[:], rec[:])
                    nc.vector.tensor_mul(atn[:, h, :], ops[:], rb[:])

                # ---- D: partial o_proj for this block's 512 tokens ----
                for T in range(TB // 128):
                    for n in range(HIDDEN // 512):
                        ps = psp.tile([128, 512], f32, tag="qps", bufs=4,
                                      name=f"ops_{g}_{T}_{n}")
                        for h2 in range(HPC):
                            nc.tensor.matmul(
                                ps[:],
                                atn[:, h2, T * 128:(T + 1) * 128],
                                wo_sb[:, h2, n * 512:(n + 1) * 512],
                                start=(h2 == 0), stop=(h2 == HPC - 1))
                        osb = ring.tile([128, 512], bf16, tag="osb", bufs=3,
                                        name=f"osb{g}_{T}_{n}")
                        nc.vector.tensor_copy(osb[:], ps[:])
                        nc.sync.dma_start(
                            outp[c0 + T * 128: c0 + (T + 1) * 128,
                                 n * 512:(n + 1) * 512], osb[:])

    nc.compile()
    return nc


def make_in_maps(hidden_states, Wq, Wkr, Wdk, Wupk, Wupv, Wo):
    """Host-side sharding + layout prep (bf16). Returns per-core input dicts."""
    import ml_dtypes
    bf16 = ml_dtypes.bfloat16
    scale = 1.0 / np.sqrt(np.float32(HEAD_DIM))

    hidT = np.ascontiguousarray(
        np.asarray(hidden_states, np.float32).reshape(NT, HIDDEN).T).astype(bf16)

    cos_t, sin_t = _rope_tables(S)                     # [128, S] f32
    qcos = (cos_t * scale).astype(bf16)
    qsin = (np.concatenate([-sin_t[0:64], sin_t[64:128]], axis=0) * scale).astype(bf16)
    kcos = np.zeros((128, S), np.float32)
    ksin = np.zeros((128, S), np.float32)
    kcos[64:96] = cos_t[0:32]
    kcos[96:128] = cos_t[64:96]
    ksin[64:96] = -sin_t[0:32]
    ksin[96:128] = sin_t[64:96]
    kcos = kcos.astype(bf16)
    ksin = ksin.astype(bf16)

    k_idx = np.arange(128)[:, None]
    q_idx = np.arange(TB)[None, :]
    masks = np.stack(
        [(q_idx >= j * 128 + k_idx).astype(np.float32) for j in range(NJ)],
        axis=1).astype(bf16)                           # [128, NJ, TB]

    Wq = np.asarray(Wq, np.float32)
    Wkr = np.asarray(Wkr, np.float32)
    Wdk = np.asarray(Wdk, np.float32)
    Wupk = np.asarray(Wupk, np.float32)
    Wupv = np.asarray(Wupv, np.float32)
    Wo = np.asarray(Wo, np.float32)

    in_maps = []
    for c in range(NCORES):
        wq_t = np.ascontiguousarray(Wq[QR * c:QR * (c + 1)].T).astype(bf16)
        wdkkr_t = np.ascontiguousarray(
            np.concatenate([Wdk[LSH * c:LSH * (c + 1)],
                            Wkr[KRR * c:KRR * (c + 1)]], axis=0).T).astype(bf16)
        wupk_t = np.ascontiguousarray(Wupk[KRR * c:KRR * (c + 1)].T).astype(bf16)
        wupv_t = np.ascontiguousarray(
            Wupv[HEAD_DIM * c:HEAD_DIM * (c + 1)].T).astype(bf16)
        wo_t = np.ascontiguousarray(Wo[:, QR * c:QR * (c + 1)].T).astype(bf16)
        in_maps.append({
            "hidT": hidT, "wq_t": wq_t, "wdkkr_t": wdkkr_t,
            "wupk_t": wupk_t, "wupv_t": wupv_t, "wo_t": wo_t,
            "qcos": qcos, "qsin": qsin, "kcos": kcos, "ksin": ksin,
            "masks": masks,
        })
    return in_maps


_NC_CACHE = {}


def _get_program(key=0):
    if key not in _NC_CACHE:
        _NC_CACHE[key] = build_program()
    return _NC_CACHE[key]


def finish_output(res):
    out = res.results[0]["out_part"].astype(np.float32)
    for i in range(1, NCORES):
        out = out + res.results[i]["out_part"].astype(np.float32)
    return out.reshape(B, S, HIDDEN).astype(np.float32)


def kernel(hidden_states, Wq, Wkr, Wdk, Wupk, Wupv, Wo):
    from concourse.bass_utils import run_bass_kernel_spmd

    in_maps = make_in_maps(hidden_states, Wq, Wkr, Wdk, Wupk, Wupv, Wo)
    nc = _get_program()
    res = run_bass_kernel_spmd(nc, in_maps, list(range(NCORES)))
    return finish_output(res)


# revision 7
# speedup vs baseline: 1.3526x; 1.2133x over previous
"""MLA (CustomLlamaMLAForInfer) Trainium2 Bass kernel, v3.

Sharding: tensor-parallel over heads across 8 NeuronCores. Core c owns
kv-head c and q-heads [4c, 4c+4). Every core sees the full token stream
(B*S = 4096 tokens). The shared low-rank latent (c_kv, 512 dims) is
*sharded*: core c computes latent dims [64c, 64c+64) for all tokens and
an AllGather rebuilds the full latent on every core. o_proj is
row-sharded; the host sums the 8 partial [4096, 4096] outputs.

All matmuls in bf16 (inputs pre-converted host-side), fp32 PSUM.
One streaming TileContext; PE executes strictly in emission order:

  A(g), g=0..7 (512-token blocks): qT = Wq_shard @ hid.T (rope folded
     in at evict, kept in SBUF), [c_kv shard; k_rope shard] fused
     matmul; c_kv shard -> DRAM -> AllGather (overlapped with later
     A blocks); k_rope roped+scattered into SBUF kT.
  B(g): k_nope/v of the core's kv head from the gathered latent.
  C(g): causal attention for q-block g, 4 q-heads. k-tiles processed
     in PAIRS: two 512-col score matmuls into one 2-bank [128,1024]
     PSUM tile, ONE exp (ScalarE) per pair, paired causal masks,
     v.T@p + ones-matmul denominators, one-pair software pipeline.
  D(g): partial o_proj; PSUM evicted straight to DRAM via DMA (f32).

PSUM (8 banks): big [128,1024] x2 (A q-pairs / C score-pairs),
mid [128,512] x2 (A dk+kr / B knope,v / C out-accum / D o_proj),
sum [1,512] x2 (softmax denominators).
"""

import numpy as np

HIDDEN = 4096
N_HEADS = 32
KV_HEADS = 8
HEAD_DIM = 128
LOW_RANK = 64
TOP_K_ROPE = 32
ROPE_THETA = 10000.0
B, S = 2, 2048
NCORES = 8
HPC = N_HEADS // NCORES          # q heads per core = 4
QR = HPC * HEAD_DIM              # q rows per core = 512
CD = LOW_RANK * KV_HEADS         # latent dim = 512
LSH = CD // NCORES               # latent shard per core = 64
KRR = 2 * TOP_K_ROPE             # rope rows per kv head = 64
NT = B * S                       # total tokens = 4096
TB = 512                         # token block
NG = NT // TB                    # token blocks = 8
HT = HIDDEN // 128               # hidden k-tiles = 32
LT = CD // 128                   # latent k-tiles = 4
NJ = TB // 128                   # diag mask variants = 4


def _rope_tables(seq_len):
    inv = 1.0 / (ROPE_THETA ** (np.arange(0, HEAD_DIM, 2, dtype=np.float32) / HEAD_DIM))
    pos = np.arange(seq_len, dtype=np.float32)
    fr = np.outer(pos, inv)
    emb = np.concatenate([fr, fr], axis=-1)          # [S, 128]
    return (np.cos(emb).T.astype(np.float32),        # [128, S]
            np.sin(emb).T.astype(np.float32))


def build_program(trace_sim=False):
    from concourse import bacc, tile, mybir
    import concourse.bass as bass

    f32 = mybir.dt.float32
    bf16 = mybir.dt.bfloat16
    MS = bass.MemorySpace
    EXP = mybir.ActivationFunctionType.Exp

    nc = bacc.Bacc("TRN2", target_bir_lowering=False, debug=False,
                   num_devices=NCORES)

    def din(name, shape):
        return nc.dram_tensor(name, shape, bf16, kind="ExternalInput").ap()

    hidT = din("hidT", [HIDDEN, NT])
    wq = din("wq_t", [HIDDEN, QR])
    wdkkr = din("wdkkr_t", [HIDDEN, 128])
    wupk = din("wupk_t", [CD, KRR])
    wupv = din("wupv_t", [CD, HEAD_DIM])
    wo = din("wo_t", [QR, HIDDEN])
    qcos = din("qcos", [128, S])
    qsin = din("qsin", [128, S])
    kcos = din("kcos", [128, S])     # rows 64:128 hold the values
    ksin = din("ksin", [128, S])     # rows 64:128 hold the values
    masks2 = din("masks2", [128, NJ // 2, 2 * TB])
    outp = nc.dram_tensor("out_part", [NT, HIDDEN], bf16,
                          kind="ExternalOutput").ap()

    rg = [list(range(NCORES))]

    with tile.TileContext(nc, trace_sim=trace_sim) as tc:
        with tc.tile_pool(name="pers", bufs=1) as pers, \
             tc.tile_pool(name="ring", bufs=2) as ring, \
             tc.tile_pool(name="dram", bufs=8, space="DRAM") as dram, \
             tc.tile_pool(name="ps", bufs=1, space=MS.PSUM) as psp:

            # ---------------- persistent SBUF ----------------
            wq_sb = pers.tile([128, HT, QR], bf16, tag="wq")
            nc.sync.dma_start(wq_sb[:], wq.rearrange("(t p) w -> p t w", p=128))
            wdkkr_sb = pers.tile([128, HT, 128], bf16, tag="wdkkr")
            nc.sync.dma_start(wdkkr_sb[:], wdkkr.rearrange("(t p) w -> p t w", p=128))
            wupk_sb = pers.tile([128, LT, KRR], bf16, tag="wupk")
            nc.sync.dma_start(wupk_sb[:], wupk.rearrange("(t p) w -> p t w", p=128))
            wupv_sb = pers.tile([128, LT, HEAD_DIM], bf16, tag="wupv")
            nc.sync.dma_start(wupv_sb[:], wupv.rearrange("(t p) w -> p t w", p=128))
            wo_sb = pers.tile([128, HPC, HIDDEN], bf16, tag="wo")
            nc.sync.dma_start(wo_sb[:], wo.rearrange("(h p) w -> p h w", p=128))
            qcos_sb = pers.tile([128, S], bf16, tag="qcos")
            nc.sync.dma_start(qcos_sb[:], qcos)
            qsin_sb = pers.tile([128, S], bf16, tag="qsin")
            nc.sync.dma_start(qsin_sb[:], qsin)
            kcos_sb = pers.tile([128, S], bf16, tag="kcos")
            nc.sync.dma_start(kcos_sb[:], kcos)
            ksin_sb = pers.tile([128, S], bf16, tag="ksin")
            nc.sync.dma_start(ksin_sb[:], ksin)
            masks_sb = pers.tile([128, NJ // 2, 2 * TB], bf16, tag="masks")
            nc.sync.dma_start(masks_sb[:], masks2)
            ones_sb = pers.tile([128, 1], bf16, tag="ones")
            nc.vector.memset(ones_sb[:], 1.0)

            qT_sb = pers.tile([128, HPC, NT], bf16, tag="qT")
            kT_sb = pers.tile([128, NT], bf16, tag="kT")
            v_sb = pers.tile([128, NT // 128, HEAD_DIM], bf16, tag="v")

            gaths = []

            # ================ phase A: projections ================
            for g in range(NG):
                c0, c1 = g * TB, (g + 1) * TB
                p0 = (g % (S // TB)) * TB          # table col (per batch)
                p1 = p0 + TB
                qp2 = [psp.tile([128, 2 * TB], f32, tag="big", bufs=2,
                                name=f"qp2_{g}_{i}") for i in range(2)]
                dkp = psp.tile([128, TB], f32, tag="mid", bufs=2,
                               name=f"dkp{g}")
                for t in range(HT):
                    ht = ring.tile([128, TB], bf16, tag="hid", bufs=10,
                                   name=f"hid{g}_{t}")
                    nc.sync.dma_start(ht[:], hidT[t * 128:(t + 1) * 128, c0:c1])
                    for m in range(HPC):
                        nc.tensor.matmul(
                            qp2[m // 2][:, (m % 2) * TB:(m % 2 + 1) * TB],
                            wq_sb[:, t, m * 128:(m + 1) * 128],
                            ht[:],
                            start=(t == 0), stop=(t == HT - 1))
                    nc.tensor.matmul(dkp[:], wdkkr_sb[:, t, :], ht[:],
                                     start=(t == 0), stop=(t == HT - 1))
                # ---- evict q heads with rope ----
                for m in range(HPC):
                    src = qp2[m // 2][:, (m % 2) * TB:(m % 2 + 1) * TB]
                    raw = ring.tile([128, TB], bf16, tag="raw", name=f"raw{g}_{m}")
                    nc.scalar.copy(raw[:], src)
                    rot = ring.tile([128, TB], bf16, tag="rot", name=f"rot{g}_{m}")
                    nc.sync.dma_start(rot[0:64, :], raw[64:128, :])
                    nc.sync.dma_start(rot[64:128, :], raw[0:64, :])
                    qsb = ring.tile([128, TB], bf16, tag="qsb", name=f"qsb{g}_{m}")
                    nc.vector.tensor_mul(qsb[:], raw[:], qcos_sb[:, p0:p1])
                    nc.vector.tensor_mul(rot[:], rot[:], qsin_sb[:, p0:p1])
                    nc.vector.tensor_add(qT_sb[:, m, c0:c1], qsb[:], rot[:])
                # ---- evict latent shard + k rope ----
                dka = ring.tile([128, TB], bf16, tag="dka", name=f"dka{g}")
                nc.scalar.copy(dka[:], dkp[:])
                ckv_my = dram.tile([LSH, TB], bf16, tag="ckv_my", name=f"ckvmy{g}")
                nc.sync.dma_start(ckv_my[:], dka[0:LSH, :])
                gath = dram.tile([CD, TB], bf16, tag="gath", addr_space="Shared",
                                 name=f"gath{g}")
                nc.gpsimd.collective_compute(
                    "AllGather", mybir.AluOpType.bypass, replica_groups=rg,
                    ins=[ckv_my.opt()], outs=[gath.opt()])
                gaths.append(gath)
                # rope rows live at partitions 64:128
                rotk = ring.tile([128, TB], bf16, tag="rotk", name=f"rotk{g}")
                nc.sync.dma_start(rotk[64:96, :], dka[96:128, :])
                nc.sync.dma_start(rotk[96:128, :], dka[64:96, :])
                ktm = ring.tile([128, TB], bf16, tag="ktm", name=f"ktm{g}")
                nc.vector.tensor_mul(ktm[64:128, :], dka[64:128, :],
                                     kcos_sb[64:128, p0:p1])
                nc.vector.tensor_mul(rotk[64:128, :], rotk[64:128, :],
                                     ksin_sb[64:128, p0:p1])
                nc.vector.tensor_add(ktm[64:128, :], ktm[64:128, :],
                                     rotk[64:128, :])
                nc.sync.dma_start(kT_sb[0:32, c0:c1], ktm[64:96, :])
                nc.sync.dma_start(kT_sb[64:96, c0:c1], ktm[96:128, :])

            # ================ phases B/C/D per block ================
            for g in range(NG):
                c0, c1 = g * TB, (g + 1) * TB
                b, qb = g // (S // TB), g % (S // TB)
                off = b * S
                # ---- B: k_nope + v from gathered latent ----
                cb = ring.tile([128, LT, TB], bf16, tag="cb", name=f"cb{g}")
                nc.sync.dma_start(cb[:], gaths[g].rearrange("(l p) w -> p l w", p=128))
                kcp = psp.tile([KRR, TB], f32, tag="mid", bufs=2, name=f"kcp{g}")
                for lt in range(LT):
                    nc.tensor.matmul(kcp[:], wupk_sb[:, lt, :], cb[:, lt, :],
                                     start=(lt == 0), stop=(lt == LT - 1))
                kcs = ring.tile([KRR, TB], bf16, tag="kcs", name=f"kcs{g}")
                nc.scalar.copy(kcs[:], kcp[:])
                nc.sync.dma_start(kT_sb[32:64, c0:c1], kcs[0:32, :])
                nc.sync.dma_start(kT_sb[96:128, c0:c1], kcs[32:64, :])
                for tt in range(TB // 128):
                    vp = psp.tile([128, HEAD_DIM], f32, tag="mid", bufs=2,
                                  name=f"vp{g}_{tt}")
                    for lt in range(LT):
                        nc.tensor.matmul(vp[:],
                                         cb[:, lt, tt * 128:(tt + 1) * 128],
                                         wupv_sb[:, lt, :],
                                         start=(lt == 0), stop=(lt == LT - 1))
                    nc.scalar.copy(v_sb[:, g * (TB // 128) + tt, :], vp[:])

                # ---- C: attention for q-block (b, qb), 4 heads ----
                atn = ring.tile([128, HPC, TB], bf16, tag="atn", name=f"atn{g}")
                nkt = (qb + 1) * NJ
                npair = nkt // 2
                for h in range(HPC):
                    qsl = qT_sb[:, h, off + qb * TB: off + (qb + 1) * TB]
                    ops = psp.tile([128, TB], f32, tag="mid", bufs=2,
                                   name=f"ops{g}_{h}")
                    sps = psp.tile([1, TB], f32, tag="sum", bufs=2,
                                   name=f"sps{g}_{h}")
                    pts = [None, None]
                    for p in range(npair + 1):
                        if p < npair:
                            sc2 = psp.tile([128, 2 * TB], f32, tag="big", bufs=2,
                                           name=f"sc2_{g}_{h}_{p}")
                            for u in range(2):
                                kt = 2 * p + u
                                nc.tensor.matmul(
                                    sc2[:, u * TB:(u + 1) * TB],
                                    kT_sb[:, off + kt * 128: off + (kt + 1) * 128],
                                    qsl,
                                    start=True, stop=True)
                            pt2 = ring.tile([128, 2 * TB], bf16, tag="pt2", bufs=3,
                                            name=f"pt2_{g}_{h}_{p}")
                            nc.scalar.activation(pt2[:], sc2[:], EXP)
                            jp = p - qb * (NJ // 2)
                            if jp >= 0:
                                nc.vector.tensor_mul(pt2[:], pt2[:],
                                                     masks_sb[:, jp, :])
                            pts[p % 2] = pt2
                        pp = p - 1
                        if pp >= 0:
                            ppt = pts[pp % 2]
                            for u in range(2):
                                kt = 2 * pp + u
                                nc.tensor.matmul(
                                    ops[:],
                                    v_sb[:, b * (S // 128) + kt, :],
                                    ppt[:, u * TB:(u + 1) * TB],
                                    start=(kt == 0), stop=(kt == nkt - 1))
                                nc.tensor.matmul(
                                    sps[:], ones_sb[:],
                                    ppt[:, u * TB:(u + 1) * TB],
                                    start=(kt == 0), stop=(kt == nkt - 1))
                    rec = ring.tile([1, TB], f32, tag="rec", name=f"rec{g}_{h}")
                    nc.vector.reciprocal_approx_fast(rec[:], sps[:])
                    rbs = ring.tile([128, TB], f32, tag="rbs", name=f"rbs{g}_{h}")
                    nc.gpsimd.partition_broadcast(rbs[:], rec[:])
                    nc.vector.tensor_mul(atn[:, h, :], ops[:], rbs[:])

                # ---- D: partial o_proj ----
                for T in range(TB // 128):
                    for n in range(HIDDEN // 512):
                        ps = psp.tile([128, 512], f32, tag="mid", bufs=2,
                                      name=f"od_{g}_{T}_{n}")
                        for h2 in range(HPC):
                            nc.tensor.matmul(
                                ps[:],
                                atn[:, h2, T * 128:(T + 1) * 128],
                                wo_sb[:, h2, n * 512:(n + 1) * 512],
                                start=(h2 == 0), stop=(h2 == HPC - 1))
                        osb = ring.tile([128, 512], bf16, tag="osb", bufs=4,
                                        name=f"osb{g}_{T}_{n}")
                        if n % 2 == 0:
                            nc.scalar.copy(osb[:], ps[:])
                        else:
                            nc.vector.tensor_copy(osb[:], ps[:])
                        nc.sync.dma_start(
                            outp[c0 + T * 128: c0 + (T + 1) * 128,
                                 n * 512:(n + 1) * 512], osb[:])

    nc.compile()
    return nc


def make_in_maps(hidden_states, Wq, Wkr, Wdk, Wupk, Wupv, Wo):
    """Host-side sharding + layout prep (bf16). Returns per-core input dicts."""
    import ml_dtypes
    bf16 = ml_dtypes.bfloat16
    scale = 1.0 / np.sqrt(np.float32(HEAD_DIM))

    hidT = np.ascontiguousarray(
        np.asarray(hidden_states, np.float32).reshape(NT, HIDDEN).T).astype(bf16)

    cos_t, sin_t = _rope_tables(S)                     # [128, S] f32
    qcos = (cos_t * scale).astype(bf16)
    qsin = (np.concatenate([-sin_t[0:64], sin_t[64:128]], axis=0) * scale).astype(bf16)
    kcos = np.zeros((128, S), np.float32)
    ksin = np.zeros((128, S), np.float32)
    kcos[64:96] = cos_t[0:32]
    kcos[96:128] = cos_t[64:96]
    ksin[64:96] = -sin_t[0:32]
    ksin[96:128] = sin_t[64:96]
    kcos = kcos.astype(bf16)
    ksin = ksin.astype(bf16)

    k_idx = np.arange(128)[:, None]
    q_idx = np.arange(TB)[None, :]
    m1 = np.stack(
        [(q_idx >= j * 128 + k_idx).astype(np.float32) for j in range(NJ)],
        axis=1)                                        # [128, NJ, TB]
    masks2 = np.concatenate(
        [np.stack([m1[:, 0], m1[:, 2]], axis=1),
         np.stack([m1[:, 1], m1[:, 3]], axis=1)], axis=2).astype(bf16)
    # masks2[:, i, 0:TB] = mask_{2i}, masks2[:, i, TB:2TB] = mask_{2i+1}

    Wq = np.asarray(Wq, np.float32)
    Wkr = np.asarray(Wkr, np.float32)
    Wdk = np.asarray(Wdk, np.float32)
    Wupk = np.asarray(Wupk, np.float32)
    Wupv = np.asarray(Wupv, np.float32)
    Wo = np.asarray(Wo, np.float32)

    in_maps = []
    for c in range(NCORES):
        wq_t = np.ascontiguousarray(Wq[QR * c:QR * (c + 1)].T).astype(bf16)
        wdkkr_t = np.ascontiguousarray(
            np.concatenate([Wdk[LSH * c:LSH * (c + 1)],
                            Wkr[KRR * c:KRR * (c + 1)]], axis=0).T).astype(bf16)
        wupk_t = np.ascontiguousarray(Wupk[KRR * c:KRR * (c + 1)].T).astype(bf16)
        wupv_t = np.ascontiguousarray(
            Wupv[HEAD_DIM * c:HEAD_DIM * (c + 1)].T).astype(bf16)
        wo_t = np.ascontiguousarray(Wo[:, QR * c:QR * (c + 1)].T).astype(bf16)
        in_maps.append({
            "hidT": hidT, "wq_t": wq_t, "wdkkr_t": wdkkr_t,
            "wupk_t": wupk_t, "wupv_t": wupv_t, "wo_t": wo_t,
            "qcos": qcos, "qsin": qsin, "kcos": kcos, "ksin": ksin,
            "masks2": masks2,
        })
    return in_maps


_NC_CACHE = {}


def _get_program(key=0):
    if key not in _NC_CACHE:
        _NC_CACHE[key] = build_program()
    return _NC_CACHE[key]


def finish_output(res):
    out = res.results[0]["out_part"].astype(np.float32)
    for i in range(1, NCORES):
        out = out + res.results[i]["out_part"].astype(np.float32)
    return out.reshape(B, S, HIDDEN).astype(np.float32)


def kernel(hidden_states, Wq, Wkr, Wdk, Wupk, Wupv, Wo):
    from concourse.bass_utils import run_bass_kernel_spmd

    in_maps = make_in_maps(hidden_states, Wq, Wkr, Wdk, Wupk, Wupv, Wo)
    nc = _get_program()
    res = run_bass_kernel_spmd(nc, in_maps, list(range(NCORES)))
    return finish_output(res)


# revision 17
# speedup vs baseline: 1.3931x; 1.0300x over previous
"""MLA (CustomLlamaMLAForInfer) Trainium2 Bass kernel, v3.

Sharding: tensor-parallel over heads across 8 NeuronCores. Core c owns
kv-head c and q-heads [4c, 4c+4). Every core sees the full token stream
(B*S = 4096 tokens). The shared low-rank latent (c_kv, 512 dims) is
*sharded*: core c computes latent dims [64c, 64c+64) for all tokens and
an AllGather rebuilds the full latent on every core. o_proj is
row-sharded; the host sums the 8 partial [4096, 4096] outputs.

All matmuls in bf16 (inputs pre-converted host-side), fp32 PSUM.
One streaming TileContext; PE executes strictly in emission order:

  A(g), g=0..7 (512-token blocks): qT = Wq_shard @ hid.T (rope folded
     in at evict, kept in SBUF), [c_kv shard; k_rope shard] fused
     matmul; c_kv shard -> DRAM -> AllGather (overlapped with later
     A blocks); k_rope roped+scattered into SBUF kT.
  B(g): k_nope/v of the core's kv head from the gathered latent.
  C(g): causal attention for q-block g, 4 q-heads. k-tiles processed
     in PAIRS: two 512-col score matmuls into one 2-bank [128,1024]
     PSUM tile, ONE exp (ScalarE) per pair, paired causal masks,
     v.T@p + ones-matmul denominators, one-pair software pipeline.
  D(g): partial o_proj; PSUM evicted straight to DRAM via DMA (f32).

PSUM (8 banks): big [128,1024] x2 (A q-pairs / C score-pairs),
mid [128,512] x2 (A dk+kr / B knope,v / C out-accum / D o_proj),
sum [1,512] x2 (softmax denominators).
"""

import numpy as np

HIDDEN = 4096
N_HEADS = 32
KV_HEADS = 8
HEAD_DIM = 128
LOW_RANK = 64
TOP_K_ROPE = 32
ROPE_THETA = 10000.0
B, S = 2, 2048
NCORES = 8
HPC = N_HEADS // NCORES          # q heads per core = 4
QR = HPC * HEAD_DIM              # q rows per core = 512
CD = LOW_RANK * KV_HEADS         # latent dim = 512
LSH = CD // NCORES               # latent shard per core = 64
KRR = 2 * TOP_K_ROPE             # rope rows per kv head = 64
NT = B * S                       # total tokens = 4096
TB = 512                         # token block
NG = NT // TB                    # token blocks = 8
HT = HIDDEN // 128               # hidden k-tiles = 32
LT = CD // 128                   # latent k-tiles = 4
NJ = TB // 128                   # diag mask variants = 4


def _rope_tables(seq_len):
    inv = 1.0 / (ROPE_THETA ** (np.arange(0, HEAD_DIM, 2, dtype=np.float32) / HEAD_DIM))
    pos = np.arange(seq_len, dtype=np.float32)
    fr = np.outer(pos, inv)
    emb = np.concatenate([fr, fr], axis=-1)          # [S, 128]
    return (np.cos(emb).T.astype(np.float32),        # [128, S]
            np.sin(emb).T.astype(np.float32))


def build_program(trace_sim=False):
    from concourse import bacc, tile, mybir
    import concourse.bass as bass

    f32 = mybir.dt.float32
    F32R = mybir.dt.float32r
    bf16 = mybir.dt.bfloat16
    MS = bass.MemorySpace
    EXP = mybir.ActivationFunctionType.Exp

    nc = bacc.Bacc("TRN2", target_bir_lowering=False, debug=False,
                   num_devices=NCORES)

    def din(name, shape):
        return nc.dram_tensor(name, shape, bf16, kind="ExternalInput").ap()

    hidT = din("hidT", [HIDDEN, NT])
    wq = din("wq_t", [HIDDEN, QR])
    wdkkr = din("wdkkr_t", [HIDDEN, 128])
    wupk = din("wupk_t", [CD, KRR])
    wupv = din("wupv_t", [CD, HEAD_DIM])
    wo = din("wo_t", [QR, HIDDEN])
    qcos = din("qcos", [128, S])
    qsin = din("qsin", [128, S])
    kcos = din("kcos", [128, S])     # rows 64:128 hold the values
    ksin = din("ksin", [128, S])     # rows 64:128 hold the values
    masks2 = din("masks2", [128, NJ // 2, 2 * TB])
    outp = nc.dram_tensor("out_part", [NT, HIDDEN], bf16,
                          kind="ExternalOutput").ap()

    rg = [list(range(NCORES))]

    with tile.TileContext(nc, trace_sim=trace_sim) as tc:
        with tc.tile_pool(name="pers", bufs=1) as pers, \
             tc.tile_pool(name="ring", bufs=2) as ring, \
             tc.tile_pool(name="dram", bufs=8, space="DRAM") as dram, \
             tc.tile_pool(name="ps", bufs=1, space=MS.PSUM) as psp:

            # ---------------- persistent SBUF ----------------
            # A-phase-critical loads first; wq split so A(0) starts after
            # its first chunk lands.
            wq_sb = pers.tile([128, HT, QR], bf16, tag="wq")
            wqr = wq.rearrange("(t p) w -> p t w", p=128)
            for ch in range(4):
                t0, t1 = ch * (HT // 4), (ch + 1) * (HT // 4)
                nc.sync.dma_start(wq_sb[:, t0:t1, :], wqr[:, t0:t1, :])
            wdkkr_sb = pers.tile([128, HT, 128], bf16, tag="wdkkr")
            nc.sync.dma_start(wdkkr_sb[:], wdkkr.rearrange("(t p) w -> p t w", p=128))
            qcos_sb = pers.tile([128, S], bf16, tag="qcos")
            nc.sync.dma_start(qcos_sb[:], qcos)
            qsin_sb = pers.tile([128, S], bf16, tag="qsin")
            nc.sync.dma_start(qsin_sb[:], qsin)
            kcos_sb = pers.tile([128, S], bf16, tag="kcos")
            nc.sync.dma_start(kcos_sb[:], kcos)
            ksin_sb = pers.tile([128, S], bf16, tag="ksin")
            nc.sync.dma_start(ksin_sb[:], ksin)
            # BCD-phase loads (not start-critical)
            wupk_sb = pers.tile([128, LT, KRR], bf16, tag="wupk")
            nc.sync.dma_start(wupk_sb[:], wupk.rearrange("(t p) w -> p t w", p=128))
            wupv_sb = pers.tile([128, LT, HEAD_DIM], bf16, tag="wupv")
            nc.sync.dma_start(wupv_sb[:], wupv.rearrange("(t p) w -> p t w", p=128))
            wo_sb = pers.tile([128, HPC, HIDDEN], bf16, tag="wo")
            nc.sync.dma_start(wo_sb[:], wo.rearrange("(h p) w -> p h w", p=128))
            masks_sb = pers.tile([128, NJ // 2, 2 * TB], bf16, tag="masks")
            nc.sync.dma_start(masks_sb[:], masks2)
            ones_f32 = pers.tile([128, 1], f32, tag="ones_f32")
            nc.vector.memset(ones_f32[:], 1.0)
            ones32_sb = pers.tile([128, 1], F32R, tag="ones32")
            nc.scalar.copy(ones32_sb[:], ones_f32[:])

            qT_sb = pers.tile([128, HPC, NT], bf16, tag="qT")
            kT_sb = pers.tile([128, NT], bf16, tag="kT")
            v_sb = pers.tile([128, NT // 128, HEAD_DIM], bf16, tag="v")

            gaths = []

            # ================ phase A: projections ================
            for g in range(NG):
                c0, c1 = g * TB, (g + 1) * TB
                p0 = (g % (S // TB)) * TB          # table col (per batch)
                p1 = p0 + TB
                qp2 = [psp.tile([128, 2 * TB], f32, tag="big", bufs=2,
                                name=f"qp2_{g}_{i}") for i in range(2)]
                dkp = psp.tile([128, TB], f32, tag="mid", bufs=2,
                               name=f"dkp{g}")
                for t in range(HT):
                    ht = ring.tile([128, TB], bf16, tag="hid", bufs=8,
                                   name=f"hid{g}_{t}")
                    nc.sync.dma_start(ht[:], hidT[t * 128:(t + 1) * 128, c0:c1])
                    for m in range(HPC):
                        nc.tensor.matmul(
                            qp2[m // 2][:, (m % 2) * TB:(m % 2 + 1) * TB],
                            wq_sb[:, t, m * 128:(m + 1) * 128],
                            ht[:],
                            start=(t == 0), stop=(t == HT - 1))
                    nc.tensor.matmul(dkp[:], wdkkr_sb[:, t, :], ht[:],
                                     start=(t == 0), stop=(t == HT - 1))
                # ---- evict q heads with rope ----
                for m in range(HPC):
                    src = qp2[m // 2][:, (m % 2) * TB:(m % 2 + 1) * TB]
                    raw = ring.tile([128, TB], bf16, tag="raw", name=f"raw{g}_{m}")
                    nc.scalar.copy(raw[:], src)
                    rot = ring.tile([128, TB], bf16, tag="rot", name=f"rot{g}_{m}")
                    nc.sync.dma_start(rot[0:64, :], raw[64:128, :])
                    nc.sync.dma_start(rot[64:128, :], raw[0:64, :])
                    qsb = ring.tile([128, TB], bf16, tag="qsb", name=f"qsb{g}_{m}")
                    nc.vector.tensor_mul(qsb[:], raw[:], qcos_sb[:, p0:p1])
                    nc.vector.tensor_mul(rot[:], rot[:], qsin_sb[:, p0:p1])
                    nc.vector.tensor_add(qT_sb[:, m, c0:c1], qsb[:], rot[:])
                # ---- evict latent shard + k rope ----
                dka = ring.tile([128, TB], bf16, tag="dka", name=f"dka{g}")
                nc.scalar.copy(dka[:], dkp[:])
                ckv_my = dram.tile([LSH, TB], bf16, tag="ckv_my", name=f"ckvmy{g}")
                nc.sync.dma_start(ckv_my[:], dka[0:LSH, :])
                gath = dram.tile([CD, TB], bf16, tag="gath", addr_space="Shared",
                                 name=f"gath{g}")
                nc.gpsimd.collective_compute(
                    "AllGather", mybir.AluOpType.bypass, replica_groups=rg,
                    ins=[ckv_my.opt()], outs=[gath.opt()])
                gaths.append(gath)
                # rope rows live at partitions 64:128
                rotk = ring.tile([128, TB], bf16, tag="rotk", name=f"rotk{g}")
                nc.sync.dma_start(rotk[64:96, :], dka[96:128, :])
                nc.sync.dma_start(rotk[96:128, :], dka[64:96, :])
                ktm = ring.tile([128, TB], bf16, tag="ktm", name=f"ktm{g}")
                nc.vector.tensor_mul(ktm[64:128, :], dka[64:128, :],
                                     kcos_sb[64:128, p0:p1])
                nc.vector.tensor_mul(rotk[64:128, :], rotk[64:128, :],
                                     ksin_sb[64:128, p0:p1])
                nc.vector.tensor_add(ktm[64:128, :], ktm[64:128, :],
                                     rotk[64:128, :])
                nc.sync.dma_start(kT_sb[0:32, c0:c1], ktm[64:96, :])
                nc.sync.dma_start(kT_sb[64:96, c0:c1], ktm[96:128, :])

            # ================ phases B/C/D per block ================
            for g in range(NG):
                c0, c1 = g * TB, (g + 1) * TB
                b, qb = g // (S // TB), g % (S // TB)
                off = b * S
                # ---- B: k_nope + v from gathered latent ----
                cb = ring.tile([128, LT, TB], bf16, tag="cb", name=f"cb{g}")
                nc.sync.dma_start(cb[:], gaths[g].rearrange("(l p) w -> p l w", p=128))
                kcp = psp.tile([KRR, TB], f32, tag="mid", bufs=2, name=f"kcp{g}")
                for lt in range(LT):
                    nc.tensor.matmul(kcp[:], wupk_sb[:, lt, :], cb[:, lt, :],
                                     start=(lt == 0), stop=(lt == LT - 1))
                kcs = ring.tile([KRR, TB], bf16, tag="kcs", name=f"kcs{g}")
                nc.scalar.copy(kcs[:], kcp[:])
                nc.sync.dma_start(kT_sb[32:64, c0:c1], kcs[0:32, :])
                nc.sync.dma_start(kT_sb[96:128, c0:c1], kcs[32:64, :])
                for tt in range(TB // 128):
                    vp = psp.tile([128, HEAD_DIM], f32, tag="mid", bufs=2,
                                  name=f"vp{g}_{tt}")
                    for lt in range(LT):
                        nc.tensor.matmul(vp[:],
                                         cb[:, lt, tt * 128:(tt + 1) * 128],
                                         wupv_sb[:, lt, :],
                                         start=(lt == 0), stop=(lt == LT - 1))
                    nc.scalar.copy(v_sb[:, g * (TB // 128) + tt, :], vp[:])

                # ---- C: attention for q-block (b, qb), 4 heads ----
                atn = ring.tile([128, HPC, TB], bf16, tag="atn", name=f"atn{g}")
                nkt = (qb + 1) * NJ
                npair = nkt // 2
                for h in range(HPC):
                    qsl = qT_sb[:, h, off + qb * TB: off + (qb + 1) * TB]
                    ops = psp.tile([128, TB], f32, tag="mid", bufs=2,
                                   name=f"ops{g}_{h}")
                    acc = ring.tile([128, 2 * TB], F32R, tag="acc", bufs=2,
                                    name=f"acc{g}_{h}")
                    pts = [None, None, None]
                    # depth-2 software pipeline: scores run 2 pairs ahead
                    # of the AV accumulation.
                    for p in range(npair + 2):
                        if p < npair:
                            sc2 = psp.tile([128, 2 * TB], f32, tag="big", bufs=2,
                                           name=f"sc2_{g}_{h}_{p}")
                            for u in range(2):
                                kt = 2 * p + u
                                nc.tensor.matmul(
                                    sc2[:, u * TB:(u + 1) * TB],
                                    kT_sb[:, off + kt * 128: off + (kt + 1) * 128],
                                    qsl,
                                    start=True, stop=True)
                            pt2 = ring.tile([128, 2 * TB], bf16, tag="pt2", bufs=3,
                                            name=f"pt2_{g}_{h}_{p}")
                            nc.scalar.activation(pt2[:], sc2[:], EXP)
                            jp = p - qb * (NJ // 2)
                            if jp >= 0:
                                nc.vector.tensor_mul(pt2[:], pt2[:],
                                                     masks_sb[:, jp, :])
                            # denominator partial sums on DVE (off PE)
                            if p == 0:
                                nc.vector.tensor_copy(acc[:], pt2[:])
                            else:
                                nc.vector.tensor_add(acc[:], acc[:], pt2[:])
                            pts[p % 3] = pt2
                        pp = p - 2
                        if pp >= 0:
                            ppt = pts[pp % 3]
                            for u in range(2):
                                kt = 2 * pp + u
                                nc.tensor.matmul(
                                    ops[:],
                                    v_sb[:, b * (S // 128) + kt, :],
                                    ppt[:, u * TB:(u + 1) * TB],
                                    start=(kt == 0), stop=(kt == nkt - 1))
                    # denominators: reduce acc halves with a ones-matmul
                    sps = psp.tile([1, TB], f32, tag="sum", bufs=2,
                                   name=f"sps{g}_{h}")
                    for u in range(2):
                        nc.tensor.matmul(sps[:], ones32_sb[:],
                                         acc[:, u * TB:(u + 1) * TB],
                                         start=(u == 0), stop=(u == 1))
                    rec = ring.tile([1, TB], f32, tag="rec", name=f"rec{g}_{h}")
                    nc.vector.reciprocal_approx_fast(rec[:], sps[:])
                    rbs = ring.tile([128, TB], f32, tag="rbs", name=f"rbs{g}_{h}")
                    nc.gpsimd.partition_broadcast(rbs[:], rec[:])
                    nc.vector.tensor_mul(atn[:, h, :], ops[:], rbs[:])

                # ---- D: partial o_proj (1024-wide moving) ----
                for T in range(TB // 128):
                    for n in range(HIDDEN // 1024):
                        ps2 = psp.tile([128, 2 * TB], f32, tag="big", bufs=2,
                                       name=f"od_{g}_{T}_{n}")
                        for half in range(2):
                            for h2 in range(HPC):
                                nc.tensor.matmul(
                                    ps2[:, half * TB:(half + 1) * TB],
                                    atn[:, h2, T * 128:(T + 1) * 128],
                                    wo_sb[:, h2, n * 1024 + half * TB:
                                          n * 1024 + (half + 1) * TB],
                                    start=(h2 == 0), stop=(h2 == HPC - 1))
                        osb = ring.tile([128, 2 * TB], bf16, tag="osb", bufs=2,
                                        name=f"osb{g}_{T}_{n}")
                        if n % 2 == 0:
                            nc.scalar.copy(osb[:], ps2[:])
                        else:
                            nc.vector.tensor_copy(osb[:], ps2[:])
                        nc.sync.dma_start(
                            outp[c0 + T * 128: c0 + (T + 1) * 128,
                                 n * 1024:(n + 1) * 1024], osb[:])

    nc.compile()
    return nc


def make_in_maps(hidden_states, Wq, Wkr, Wdk, Wupk, Wupv, Wo):
    """Host-side sharding + layout prep (bf16). Returns per-core input dicts."""
    import ml_dtypes
    bf16 = ml_dtypes.bfloat16
    scale = 1.0 / np.sqrt(np.float32(HEAD_DIM))

    hidT = np.ascontiguousarray(
        np.asarray(hidden_states, np.float32).reshape(NT, HIDDEN).T).astype(bf16)

    cos_t, sin_t = _rope_tables(S)                     # [128, S] f32
    qcos = (cos_t * scale).astype(bf16)
    qsin = (np.concatenate([-sin_t[0:64], sin_t[64:128]], axis=0) * scale).astype(bf16)
    kcos = np.zeros((128, S), np.float32)
    ksin = np.zeros((128, S), np.float32)
    kcos[64:96] = cos_t[0:32]
    kcos[96:128] = cos_t[64:96]
    ksin[64:96] = -sin_t[0:32]
    ksin[96:128] = sin_t[64:96]
    kcos = kcos.astype(bf16)
    ksin = ksin.astype(bf16)

    k_idx = np.arange(128)[:, None]
    q_idx = np.arange(TB)[None, :]
    m1 = np.stack(
        [(q_idx >= j * 128 + k_idx).astype(np.float32) for j in range(NJ)],
        axis=1)                                        # [128, NJ, TB]
    masks2 = np.concatenate(
        [np.stack([m1[:, 0], m1[:, 2]], axis=1),
         np.stack([m1[:, 1], m1[:, 3]], axis=1)], axis=2).astype(bf16)
    # masks2[:, i, 0:TB] = mask_{2i}, masks2[:, i, TB:2TB] = mask_{2i+1}

    Wq = np.asarray(Wq, np.float32)
    Wkr = np.asarray(Wkr, np.float32)
    Wdk = np.asarray(Wdk, np.float32)
    Wupk = np.asarray(Wupk, np.float32)
    Wupv = np.asarray(Wupv, np.float32)
    Wo = np.asarray(Wo, np.float32)

    in_maps = []
    for c in range(NCORES):
        wq_t = np.ascontiguousarray(Wq[QR * c:QR * (c + 1)].T).astype(bf16)
        wdkkr_t = np.ascontiguousarray(
            np.concatenate([Wdk[LSH * c:LSH * (c + 1)],
                            Wkr[KRR * c:KRR * (c + 1)]], axis=0).T).astype(bf16)
        wupk_t = np.ascontiguousarray(Wupk[KRR * c:KRR * (c + 1)].T).astype(bf16)
        wupv_t = np.ascontiguousarray(
            Wupv[HEAD_DIM * c:HEAD_DIM * (c + 1)].T).astype(bf16)
        wo_t = np.ascontiguousarray(Wo[:, QR * c:QR * (c + 1)].T).astype(bf16)
        in_maps.append({
            "hidT": hidT, "wq_t": wq_t, "wdkkr_t": wdkkr_t,
            "wupk_t": wupk_t, "wupv_t": wupv_t, "wo_t": wo_t,
            "qcos": qcos, "qsin": qsin, "kcos": kcos, "ksin": ksin,
            "masks2": masks2,
        })
    return in_maps


_NC_CACHE = {}


def _get_program(key=0):
    if key not in _NC_CACHE:
        _NC_CACHE[key] = build_program()
    return _NC_CACHE[key]


def finish_output(res):
    out = res.results[0]["out_part"].astype(np.float32)
    for i in range(1, NCORES):
        out = out + res.results[i]["out_part"].astype(np.float32)
    return out.reshape(B, S, HIDDEN).astype(np.float32)


def kernel(hidden_states, Wq, Wkr, Wdk, Wupk, Wupv, Wo):
    from concourse.bass_utils import run_bass_kernel_spmd

    in_maps = make_in_maps(hidden_states, Wq, Wkr, Wdk, Wupk, Wupv, Wo)
    nc = _get_program()
    res = run_bass_kernel_spmd(nc, in_maps, list(range(NCORES)))
    return finish_output(res)


# revision 23
# speedup vs baseline: 1.4009x; 1.0056x over previous
"""MLA (CustomLlamaMLAForInfer) Trainium2 Bass kernel, v3.

Sharding: tensor-parallel over heads across 8 NeuronCores. Core c owns
kv-head c and q-heads [4c, 4c+4). Every core sees the full token stream
(B*S = 4096 tokens). The shared low-rank latent (c_kv, 512 dims) is
*sharded*: core c computes latent dims [64c, 64c+64) for all tokens and
an AllGather rebuilds the full latent on every core. o_proj is
row-sharded; the host sums the 8 partial [4096, 4096] outputs.

All matmuls in bf16 (inputs pre-converted host-side), fp32 PSUM.
One streaming TileContext; PE executes strictly in emission order:

  A(g), g=0..7 (512-token blocks): qT = Wq_shard @ hid.T (rope folded
     in at evict, kept in SBUF), [c_kv shard; k_rope shard] fused
     matmul; c_kv shard -> DRAM -> AllGather (overlapped with later
     A blocks); k_rope roped+scattered into SBUF kT.
  B(g): k_nope/v of the core's kv head from the gathered latent.
  C(g): causal attention for q-block g, 4 q-heads. k-tiles processed
     in PAIRS: two 512-col score matmuls into one 2-bank [128,1024]
     PSUM tile, ONE exp (ScalarE) per pair, paired causal masks,
     v.T@p + ones-matmul denominators, one-pair software pipeline.
  D(g): partial o_proj; PSUM evicted straight to DRAM via DMA (f32).

PSUM (8 banks): big [128,1024] x2 (A q-pairs / C score-pairs),
mid [128,512] x2 (A dk+kr / B knope,v / C out-accum / D o_proj),
sum [1,512] x2 (softmax denominators).
"""

import numpy as np

HIDDEN = 4096
N_HEADS = 32
KV_HEADS = 8
HEAD_DIM = 128
LOW_RANK = 64
TOP_K_ROPE = 32
ROPE_THETA = 10000.0
B, S = 2, 2048
NCORES = 8
HPC = N_HEADS // NCORES          # q heads per core = 4
QR = HPC * HEAD_DIM              # q rows per core = 512
CD = LOW_RANK * KV_HEADS         # latent dim = 512
LSH = CD // NCORES               # latent shard per core = 64
KRR = 2 * TOP_K_ROPE             # rope rows per kv head = 64
NT = B * S                       # total tokens = 4096
TB = 512                         # token block
NG = NT // TB                    # token blocks = 8
HT = HIDDEN // 128               # hidden k-tiles = 32
LT = CD // 128                   # latent k-tiles = 4
NJ = TB // 128                   # diag mask variants = 4


def _rope_tables(seq_len):
    inv = 1.0 / (ROPE_THETA ** (np.arange(0, HEAD_DIM, 2, dtype=np.float32) / HEAD_DIM))
    pos = np.arange(seq_len, dtype=np.float32)
    fr = np.outer(pos, inv)
    emb = np.concatenate([fr, fr], axis=-1)          # [S, 128]
    return (np.cos(emb).T.astype(np.float32),        # [128, S]
            np.sin(emb).T.astype(np.float32))


def build_program(trace_sim=False):
    from concourse import bacc, tile, mybir
    import concourse.bass as bass

    f32 = mybir.dt.float32
    F32R = mybir.dt.float32r
    bf16 = mybir.dt.bfloat16
    MS = bass.MemorySpace
    EXP = mybir.ActivationFunctionType.Exp

    nc = bacc.Bacc("TRN2", target_bir_lowering=False, debug=False,
                   num_devices=NCORES)

    def din(name, shape):
        return nc.dram_tensor(name, shape, bf16, kind="ExternalInput").ap()

    hidT = din("hidT", [HIDDEN, NT])
    wq = din("wq_t", [HIDDEN, QR])
    wdkkr = din("wdkkr_t", [HIDDEN, 128])
    wupk = din("wupk_t", [CD, KRR])
    wupv = din("wupv_t", [CD, HEAD_DIM])
    wo = din("wo_t", [QR, HIDDEN])
    qcos = din("qcos", [128, S])
    qsin = din("qsin", [128, S])
    kcos = din("kcos", [128, S])     # rows 64:128 hold the values
    ksin = din("ksin", [128, S])     # rows 64:128 hold the values
    masks2 = din("masks2", [128, NJ // 2, 2 * TB])
    outp = nc.dram_tensor("out_part", [NT, HIDDEN], bf16,
                          kind="ExternalOutput").ap()

    rg = [list(range(NCORES))]

    with tile.TileContext(nc, trace_sim=trace_sim) as tc:
        with tc.tile_pool(name="pers", bufs=1) as pers, \
             tc.tile_pool(name="ring", bufs=2) as ring, \
             tc.tile_pool(name="dram", bufs=8, space="DRAM") as dram, \
             tc.tile_pool(name="ps", bufs=1, space=MS.PSUM) as psp:

            # ---------------- persistent SBUF ----------------
            # A-phase-critical loads first; wq split so A(0) starts after
            # its first chunk lands.
            wq_sb = pers.tile([128, HT, QR], bf16, tag="wq")
            wqr = wq.rearrange("(t p) w -> p t w", p=128)
            for ch in range(4):
                t0, t1 = ch * (HT // 4), (ch + 1) * (HT // 4)
                nc.sync.dma_start(wq_sb[:, t0:t1, :], wqr[:, t0:t1, :])
            wdkkr_sb = pers.tile([128, HT, 128], bf16, tag="wdkkr")
            nc.sync.dma_start(wdkkr_sb[:], wdkkr.rearrange("(t p) w -> p t w", p=128))
            qcos_sb = pers.tile([128, S], bf16, tag="qcos")
            nc.sync.dma_start(qcos_sb[:], qcos)
            qsin_sb = pers.tile([128, S], bf16, tag="qsin")
            nc.sync.dma_start(qsin_sb[:], qsin)
            kcos_sb = pers.tile([128, S], bf16, tag="kcos")
            nc.sync.dma_start(kcos_sb[:], kcos)
            ksin_sb = pers.tile([128, S], bf16, tag="ksin")
            nc.sync.dma_start(ksin_sb[:], ksin)
            # BCD-phase loads (not start-critical)
            wupk_sb = pers.tile([128, LT, KRR], bf16, tag="wupk")
            nc.sync.dma_start(wupk_sb[:], wupk.rearrange("(t p) w -> p t w", p=128))
            wupv_sb = pers.tile([128, LT, HEAD_DIM], bf16, tag="wupv")
            nc.sync.dma_start(wupv_sb[:], wupv.rearrange("(t p) w -> p t w", p=128))
            wo_sb = pers.tile([128, HPC, HIDDEN], bf16, tag="wo")
            nc.sync.dma_start(wo_sb[:], wo.rearrange("(h p) w -> p h w", p=128))
            masks_sb = pers.tile([128, NJ // 2, 2 * TB], bf16, tag="masks")
            nc.sync.dma_start(masks_sb[:], masks2)
            ones_f32 = pers.tile([128, 1], f32, tag="ones_f32")
            nc.vector.memset(ones_f32[:], 1.0)
            ones32_sb = pers.tile([128, 1], F32R, tag="ones32")
            nc.scalar.copy(ones32_sb[:], ones_f32[:])

            qT_sb = pers.tile([128, HPC, NT], bf16, tag="qT")
            kT_sb = pers.tile([128, NT], bf16, tag="kT")
            v_sb = pers.tile([128, NT // 128, HEAD_DIM], bf16, tag="v")

            gaths = []

            # ================ phase A: projections ================
            for g in range(NG):
                c0, c1 = g * TB, (g + 1) * TB
                p0 = (g % (S // TB)) * TB          # table col (per batch)
                p1 = p0 + TB
                qp2 = [psp.tile([128, 2 * TB], f32, tag="big", bufs=2,
                                name=f"qp2_{g}_{i}") for i in range(2)]
                dkp = psp.tile([128, TB], f32, tag="mid", bufs=2,
                               name=f"dkp{g}")
                for t in range(HT):
                    ht = ring.tile([128, TB], bf16, tag="hid", bufs=10,
                                   name=f"hid{g}_{t}")
                    nc.sync.dma_start(ht[:], hidT[t * 128:(t + 1) * 128, c0:c1])
                    for m in range(HPC):
                        nc.tensor.matmul(
                            qp2[m // 2][:, (m % 2) * TB:(m % 2 + 1) * TB],
                            wq_sb[:, t, m * 128:(m + 1) * 128],
                            ht[:],
                            start=(t == 0), stop=(t == HT - 1))
                    nc.tensor.matmul(dkp[:], wdkkr_sb[:, t, :], ht[:],
                                     start=(t == 0), stop=(t == HT - 1))
                # ---- evict q heads with rope ----
                for m in range(HPC):
                    src = qp2[m // 2][:, (m % 2) * TB:(m % 2 + 1) * TB]
                    raw = ring.tile([128, TB], bf16, tag="raw", name=f"raw{g}_{m}")
                    nc.scalar.copy(raw[:], src)
                    rot = ring.tile([128, TB], bf16, tag="rot", name=f"rot{g}_{m}")
                    nc.sync.dma_start(rot[0:64, :], raw[64:128, :])
                    nc.sync.dma_start(rot[64:128, :], raw[0:64, :])
                    qsb = ring.tile([128, TB], bf16, tag="qsb", name=f"qsb{g}_{m}")
                    nc.vector.tensor_mul(qsb[:], raw[:], qcos_sb[:, p0:p1])
                    nc.vector.tensor_mul(rot[:], rot[:], qsin_sb[:, p0:p1])
                    nc.vector.tensor_add(qT_sb[:, m, c0:c1], qsb[:], rot[:])
                # ---- evict latent shard + k rope ----
                dka = ring.tile([128, TB], bf16, tag="dka", name=f"dka{g}")
                nc.scalar.copy(dka[:], dkp[:])
                ckv_my = dram.tile([LSH, TB], bf16, tag="ckv_my", name=f"ckvmy{g}")
                nc.sync.dma_start(ckv_my[:], dka[0:LSH, :])
                gath = dram.tile([CD, TB], bf16, tag="gath", addr_space="Shared",
                                 name=f"gath{g}")
                nc.gpsimd.collective_compute(
                    "AllGather", mybir.AluOpType.bypass, replica_groups=rg,
                    ins=[ckv_my.opt()], outs=[gath.opt()])
                gaths.append(gath)
                # rope rows live at partitions 64:128
                rotk = ring.tile([128, TB], bf16, tag="rotk", name=f"rotk{g}")
                nc.sync.dma_start(rotk[64:96, :], dka[96:128, :])
                nc.sync.dma_start(rotk[96:128, :], dka[64:96, :])
                ktm = ring.tile([128, TB], bf16, tag="ktm", name=f"ktm{g}")
                nc.vector.tensor_mul(ktm[64:128, :], dka[64:128, :],
                                     kcos_sb[64:128, p0:p1])
                nc.vector.tensor_mul(rotk[64:128, :], rotk[64:128, :],
                                     ksin_sb[64:128, p0:p1])
                nc.vector.tensor_add(ktm[64:128, :], ktm[64:128, :],
                                     rotk[64:128, :])
                nc.sync.dma_start(kT_sb[0:32, c0:c1], ktm[64:96, :])
                nc.sync.dma_start(kT_sb[64:96, c0:c1], ktm[96:128, :])

            # ================ phases B/C/D per block ================
            for g in range(NG):
                c0, c1 = g * TB, (g + 1) * TB
                b, qb = g // (S // TB), g % (S // TB)
                off = b * S
                # ---- B: k_nope + v from gathered latent ----
                cb = ring.tile([128, LT, TB], bf16, tag="cb", name=f"cb{g}")
                nc.sync.dma_start(cb[:], gaths[g].rearrange("(l p) w -> p l w", p=128))
                kcp = psp.tile([KRR, TB], f32, tag="mid", bufs=2, name=f"kcp{g}")
                for lt in range(LT):
                    nc.tensor.matmul(kcp[:], wupk_sb[:, lt, :], cb[:, lt, :],
                                     start=(lt == 0), stop=(lt == LT - 1))
                kcs = ring.tile([KRR, TB], bf16, tag="kcs", name=f"kcs{g}")
                nc.scalar.copy(kcs[:], kcp[:])
                nc.sync.dma_start(kT_sb[32:64, c0:c1], kcs[0:32, :])
                nc.sync.dma_start(kT_sb[96:128, c0:c1], kcs[32:64, :])
                for tt in range(TB // 128):
                    vp = psp.tile([128, HEAD_DIM], f32, tag="mid", bufs=2,
                                  name=f"vp{g}_{tt}")
                    for lt in range(LT):
                        nc.tensor.matmul(vp[:],
                                         cb[:, lt, tt * 128:(tt + 1) * 128],
                                         wupv_sb[:, lt, :],
                                         start=(lt == 0), stop=(lt == LT - 1))
                    nc.scalar.copy(v_sb[:, g * (TB // 128) + tt, :], vp[:])

                # ---- C: attention for q-block (b, qb), 4 heads ----
                atn = ring.tile([128, HPC, TB], bf16, tag="atn", name=f"atn{g}")
                nkt = (qb + 1) * NJ
                npair = nkt // 2
                for h in range(HPC):
                    qsl = qT_sb[:, h, off + qb * TB: off + (qb + 1) * TB]
                    ops = psp.tile([128, TB], f32, tag="mid", bufs=2,
                                   name=f"ops{g}_{h}")
                    acc = ring.tile([128, 2 * TB], F32R, tag="acc", bufs=2,
                                    name=f"acc{g}_{h}")
                    pts = [None, None, None]
                    # depth-2 software pipeline: scores run 2 pairs ahead
                    # of the AV accumulation.
                    for p in range(npair + 2):
                        if p < npair:
                            sc2 = psp.tile([128, 2 * TB], f32, tag="big", bufs=2,
                                           name=f"sc2_{g}_{h}_{p}")
                            for u in range(2):
                                kt = 2 * p + u
                                nc.tensor.matmul(
                                    sc2[:, u * TB:(u + 1) * TB],
                                    kT_sb[:, off + kt * 128: off + (kt + 1) * 128],
                                    qsl,
                                    start=True, stop=True)
                            pt2 = ring.tile([128, 2 * TB], bf16, tag="pt2", bufs=3,
                                            name=f"pt2_{g}_{h}_{p}")
                            nc.scalar.activation(pt2[:], sc2[:], EXP)
                            jp = p - qb * (NJ // 2)
                            if jp >= 0:
                                nc.vector.tensor_mul(pt2[:], pt2[:],
                                                     masks_sb[:, jp, :])
                            pts[p % 3] = pt2
                        pp = p - 2
                        if pp >= 0:
                            ppt = pts[pp % 3]
                            for u in range(2):
                                kt = 2 * pp + u
                                nc.tensor.matmul(
                                    ops[:],
                                    v_sb[:, b * (S // 128) + kt, :],
                                    ppt[:, u * TB:(u + 1) * TB],
                                    start=(kt == 0), stop=(kt == nkt - 1))
                            # denominator partial sums on DVE, emitted after
                            # the AV matmuls so they don't delay the next
                            # pair's mask-mul in the DVE stream
                            if pp == 0:
                                nc.vector.tensor_copy(acc[:], ppt[:])
                            else:
                                nc.vector.tensor_add(acc[:], acc[:], ppt[:])
                    # denominators: reduce acc halves with a ones-matmul
                    sps = psp.tile([1, TB], f32, tag="sum", bufs=2,
                                   name=f"sps{g}_{h}")
                    for u in range(2):
                        nc.tensor.matmul(sps[:], ones32_sb[:],
                                         acc[:, u * TB:(u + 1) * TB],
                                         start=(u == 0), stop=(u == 1))
                    rec = ring.tile([1, TB], f32, tag="rec", bufs=1,
                                    name=f"rec{g}_{h}")
                    nc.vector.reciprocal_approx_fast(rec[:], sps[:])
                    rbs = ring.tile([128, TB], f32, tag="rbs", name=f"rbs{g}_{h}")
                    nc.gpsimd.partition_broadcast(rbs[:], rec[:])
                    nc.vector.tensor_mul(atn[:, h, :], ops[:], rbs[:])

                # ---- D: partial o_proj (1024-wide moving) ----
                for T in range(TB // 128):
                    for n in range(HIDDEN // 1024):
                        ps2 = psp.tile([128, 2 * TB], f32, tag="big", bufs=2,
                                       name=f"od_{g}_{T}_{n}")
                        for half in range(2):
                            for h2 in range(HPC):
                                nc.tensor.matmul(
                                    ps2[:, half * TB:(half + 1) * TB],
                                    atn[:, h2, T * 128:(T + 1) * 128],
                                    wo_sb[:, h2, n * 1024 + half * TB:
                                          n * 1024 + (half + 1) * TB],
                                    start=(h2 == 0), stop=(h2 == HPC - 1))
                        osb = ring.tile([128, 2 * TB], bf16, tag="osb", bufs=2,
                                        name=f"osb{g}_{T}_{n}")
                        if n % 2 == 0:
                            nc.scalar.copy(osb[:], ps2[:])
                        else:
                            nc.vector.tensor_copy(osb[:], ps2[:])
                        nc.sync.dma_start(
                            outp[c0 + T * 128: c0 + (T + 1) * 128,
                                 n * 1024:(n + 1) * 1024], osb[:])

    nc.compile()
    return nc


def make_in_maps(hidden_states, Wq, Wkr, Wdk, Wupk, Wupv, Wo):
    """Host-side sharding + layout prep (bf16). Returns per-core input dicts."""
    import ml_dtypes
    bf16 = ml_dtypes.bfloat16
    scale = 1.0 / np.sqrt(np.float32(HEAD_DIM))

    hidT = np.ascontiguousarray(
        np.asarray(hidden_states, np.float32).reshape(NT, HIDDEN).T).astype(bf16)

    cos_t, sin_t = _rope_tables(S)                     # [128, S] f32
    qcos = (cos_t * scale).astype(bf16)
    qsin = (np.concatenate([-sin_t[0:64], sin_t[64:128]], axis=0) * scale).astype(bf16)
    kcos = np.zeros((128, S), np.float32)
    ksin = np.zeros((128, S), np.float32)
    kcos[64:96] = cos_t[0:32]
    kcos[96:128] = cos_t[64:96]
    ksin[64:96] = -sin_t[0:32]
    ksin[96:128] = sin_t[64:96]
    kcos = kcos.astype(bf16)
    ksin = ksin.astype(bf16)

    k_idx = np.arange(128)[:, None]
    q_idx = np.arange(TB)[None, :]
    m1 = np.stack(
        [(q_idx >= j * 128 + k_idx).astype(np.float32) for j in range(NJ)],
        axis=1)                                        # [128, NJ, TB]
    masks2 = np.concatenate(
        [np.stack([m1[:, 0], m1[:, 2]], axis=1),
         np.stack([m1[:, 1], m1[:, 3]], axis=1)], axis=2).astype(bf16)
    # masks2[:, i, 0:TB] = mask_{2i}, masks2[:, i, TB:2TB] = mask_{2i+1}

    Wq = np.asarray(Wq, np.float32)
    Wkr = np.asarray(Wkr, np.float32)
    Wdk = np.asarray(Wdk, np.float32)
    Wupk = np.asarray(Wupk, np.float32)
    Wupv = np.asarray(Wupv, np.float32)
    Wo = np.asarray(Wo, np.float32)

    in_maps = []
    for c in range(NCORES):
        wq_t = np.ascontiguousarray(Wq[QR * c:QR * (c + 1)].T).astype(bf16)
        wdkkr_t = np.ascontiguousarray(
            np.concatenate([Wdk[LSH * c:LSH * (c + 1)],
                            Wkr[KRR * c:KRR * (c + 1)]], axis=0).T).astype(bf16)
        wupk_t = np.ascontiguousarray(Wupk[KRR * c:KRR * (c + 1)].T).astype(bf16)
        wupv_t = np.ascontiguousarray(
            Wupv[HEAD_DIM * c:HEAD_DIM * (c + 1)].T).astype(bf16)
        wo_t = np.ascontiguousarray(Wo[:, QR * c:QR * (c + 1)].T).astype(bf16)
        in_maps.append({
            "hidT": hidT, "wq_t": wq_t, "wdkkr_t": wdkkr_t,
            "wupk_t": wupk_t, "wupv_t": wupv_t, "wo_t": wo_t,
            "qcos": qcos, "qsin": qsin, "kcos": kcos, "ksin": ksin,
            "masks2": masks2,
        })
    return in_maps


_NC_CACHE = {}


def _get_program(key=0):
    if key not in _NC_CACHE:
        _NC_CACHE[key] = build_program()
    return _NC_CACHE[key]


def finish_output(res):
    out = res.results[0]["out_part"].astype(np.float32)
    for i in range(1, NCORES):
        out = out + res.results[i]["out_part"].astype(np.float32)
    return out.reshape(B, S, HIDDEN).astype(np.float32)


def kernel(hidden_states, Wq, Wkr, Wdk, Wupk, Wupv, Wo):
    from concourse.bass_utils import run_bass_kernel_spmd

    in_maps = make_in_maps(hidden_states, Wq, Wkr, Wdk, Wupk, Wupv, Wo)
    nc = _get_program()
    res = run_bass_kernel_spmd(nc, in_maps, list(range(NCORES)))
    return finish_output(res)


# revision 28
# speedup vs baseline: 1.4111x; 1.0073x over previous
"""MLA (CustomLlamaMLAForInfer) Trainium2 Bass kernel, v3.

Sharding: tensor-parallel over heads across 8 NeuronCores. Core c owns
kv-head c and q-heads [4c, 4c+4). Every core sees the full token stream
(B*S = 4096 tokens). The shared low-rank latent (c_kv, 512 dims) is
*sharded*: core c computes latent dims [64c, 64c+64) for all tokens and
an AllGather rebuilds the full latent on every core. o_proj is
row-sharded; the host sums the 8 partial [4096, 4096] outputs.

All matmuls in bf16 (inputs pre-converted host-side), fp32 PSUM.
One streaming TileContext; PE executes strictly in emission order:

  A(g), g=0..7 (512-token blocks): qT = Wq_shard @ hid.T (rope folded
     in at evict, kept in SBUF), [c_kv shard; k_rope shard] fused
     matmul; c_kv shard -> DRAM -> AllGather (overlapped with later
     A blocks); k_rope roped+scattered into SBUF kT.
  B(g): k_nope/v of the core's kv head from the gathered latent.
  C(g): causal attention for q-block g, 4 q-heads. k-tiles processed
     in PAIRS: two 512-col score matmuls into one 2-bank [128,1024]
     PSUM tile, ONE exp (ScalarE) per pair, paired causal masks,
     v.T@p + ones-matmul denominators, one-pair software pipeline.
  D(g): partial o_proj; PSUM evicted straight to DRAM via DMA (f32).

PSUM (8 banks): big [128,1024] x2 (A q-pairs / C score-pairs),
mid [128,512] x2 (A dk+kr / B knope,v / C out-accum / D o_proj),
sum [1,512] x2 (softmax denominators).
"""

import numpy as np

HIDDEN = 4096
N_HEADS = 32
KV_HEADS = 8
HEAD_DIM = 128
LOW_RANK = 64
TOP_K_ROPE = 32
ROPE_THETA = 10000.0
B, S = 2, 2048
NCORES = 8
HPC = N_HEADS // NCORES          # q heads per core = 4
QR = HPC * HEAD_DIM              # q rows per core = 512
CD = LOW_RANK * KV_HEADS         # latent dim = 512
LSH = CD // NCORES               # latent shard per core = 64
KRR = 2 * TOP_K_ROPE             # rope rows per kv head = 64
NT = B * S                       # total tokens = 4096
TB = 512                         # token block
NG = NT // TB                    # token blocks = 8
HT = HIDDEN // 128               # hidden k-tiles = 32
LT = CD // 128                   # latent k-tiles = 4
NJ = TB // 128                   # diag mask variants = 4


def _rope_tables(seq_len):
    inv = 1.0 / (ROPE_THETA ** (np.arange(0, HEAD_DIM, 2, dtype=np.float32) / HEAD_DIM))
    pos = np.arange(seq_len, dtype=np.float32)
    fr = np.outer(pos, inv)
    emb = np.concatenate([fr, fr], axis=-1)          # [S, 128]
    return (np.cos(emb).T.astype(np.float32),        # [128, S]
            np.sin(emb).T.astype(np.float32))


def build_program(trace_sim=False):
    from concourse import bacc, tile, mybir
    import concourse.bass as bass

    f32 = mybir.dt.float32
    F32R = mybir.dt.float32r
    bf16 = mybir.dt.bfloat16
    MS = bass.MemorySpace
    EXP = mybir.ActivationFunctionType.Exp

    nc = bacc.Bacc("TRN2", target_bir_lowering=False, debug=False,
                   num_devices=NCORES)

    def din(name, shape):
        return nc.dram_tensor(name, shape, bf16, kind="ExternalInput").ap()

    hidT = din("hidT", [HIDDEN, NT])
    wq = din("wq_t", [HIDDEN, QR])
    wdkkr = din("wdkkr_t", [HIDDEN, 128])
    wupk = din("wupk_t", [CD, KRR])
    wupv = din("wupv_t", [CD, HEAD_DIM])
    wo = din("wo_t", [QR, HIDDEN])
    qcos = din("qcos", [128, S])
    qsin = din("qsin", [128, S])
    kcos = din("kcos", [128, S])     # rows 64:128 hold the values
    ksin = din("ksin", [128, S])     # rows 64:128 hold the values
    masks2 = din("masks2", [128, NJ // 2, 2 * TB])
    outp = nc.dram_tensor("out_part", [NT, HIDDEN], bf16,
                          kind="ExternalOutput").ap()

    rg = [list(range(NCORES))]

    with tile.TileContext(nc, trace_sim=trace_sim) as tc:
        with tc.tile_pool(name="pers", bufs=1) as pers, \
             tc.tile_pool(name="ring", bufs=2) as ring, \
             tc.tile_pool(name="dram", bufs=8, space="DRAM") as dram, \
             tc.tile_pool(name="ps", bufs=1, space=MS.PSUM) as psp:

            # ---------------- persistent SBUF ----------------
            # A-phase-critical loads first; wq split so A(0) starts after
            # its first chunk lands.
            wq_sb = pers.tile([128, HT, QR], bf16, tag="wq")
            wqr = wq.rearrange("(t p) w -> p t w", p=128)
            for ch in range(4):
                t0, t1 = ch * (HT // 4), (ch + 1) * (HT // 4)
                nc.sync.dma_start(wq_sb[:, t0:t1, :], wqr[:, t0:t1, :])
            wdkkr_sb = pers.tile([128, HT, 128], bf16, tag="wdkkr")
            nc.sync.dma_start(wdkkr_sb[:], wdkkr.rearrange("(t p) w -> p t w", p=128))
            qcos_sb = pers.tile([128, S], bf16, tag="qcos")
            nc.sync.dma_start(qcos_sb[:], qcos)
            qsin_sb = pers.tile([128, S], bf16, tag="qsin")
            nc.sync.dma_start(qsin_sb[:], qsin)
            kcos_sb = pers.tile([128, S], bf16, tag="kcos")
            nc.sync.dma_start(kcos_sb[:], kcos)
            ksin_sb = pers.tile([128, S], bf16, tag="ksin")
            nc.sync.dma_start(ksin_sb[:], ksin)
            # BCD-phase tiles (DMAs emitted after phase A so they don't
            # contend with the A-critical hid/wq loads)
            wupk_sb = pers.tile([128, LT, KRR], bf16, tag="wupk")
            wupv_sb = pers.tile([128, LT, HEAD_DIM], bf16, tag="wupv")
            wo_sb = pers.tile([128, HPC, HIDDEN], bf16, tag="wo")
            masks_sb = pers.tile([128, NJ // 2, 2 * TB], bf16, tag="masks")
            ones_f32 = pers.tile([128, 1], f32, tag="ones_f32")
            nc.vector.memset(ones_f32[:], 1.0)
            ones32_sb = pers.tile([128, 1], F32R, tag="ones32")
            nc.scalar.copy(ones32_sb[:], ones_f32[:])

            qT_sb = pers.tile([128, HPC, NT], bf16, tag="qT")
            kT_sb = pers.tile([128, NT], bf16, tag="kT")
            v_sb = pers.tile([128, NT // 128, HEAD_DIM], bf16, tag="v")

            gaths = []

            # ================ phase A: projections ================
            for g in range(NG):
                c0, c1 = g * TB, (g + 1) * TB
                p0 = (g % (S // TB)) * TB          # table col (per batch)
                p1 = p0 + TB
                qp2 = [psp.tile([128, 2 * TB], f32, tag="big", bufs=2,
                                name=f"qp2_{g}_{i}") for i in range(2)]
                dkp = psp.tile([128, TB], f32, tag="mid", bufs=2,
                               name=f"dkp{g}")
                for t in range(HT):
                    ht = ring.tile([128, TB], bf16, tag="hid", bufs=10,
                                   name=f"hid{g}_{t}")
                    nc.sync.dma_start(ht[:], hidT[t * 128:(t + 1) * 128, c0:c1])
                    for m in range(HPC):
                        nc.tensor.matmul(
                            qp2[m // 2][:, (m % 2) * TB:(m % 2 + 1) * TB],
                            wq_sb[:, t, m * 128:(m + 1) * 128],
                            ht[:],
                            start=(t == 0), stop=(t == HT - 1))
                    nc.tensor.matmul(dkp[:], wdkkr_sb[:, t, :], ht[:],
                                     start=(t == 0), stop=(t == HT - 1))
                # ---- evict q heads with rope ----
                for i in range(2):
                    raw2 = ring.tile([128, 2 * TB], bf16, tag="raw",
                                     name=f"raw{g}_{i}")
                    nc.scalar.copy(raw2[:], qp2[i][:])
                    for mh in range(2):
                        m = 2 * i + mh
                        raw = raw2[:, mh * TB:(mh + 1) * TB]
                        rot = ring.tile([128, TB], bf16, tag="rot",
                                        name=f"rot{g}_{m}")
                        nc.sync.dma_start(rot[0:64, :], raw[64:128, :])
                        nc.sync.dma_start(rot[64:128, :], raw[0:64, :])
                        qsb = ring.tile([128, TB], bf16, tag="qsb",
                                        name=f"qsb{g}_{m}")
                        nc.vector.tensor_mul(qsb[:], raw[:], qcos_sb[:, p0:p1])
                        nc.vector.tensor_mul(rot[:], rot[:], qsin_sb[:, p0:p1])
                        nc.vector.tensor_add(qT_sb[:, m, c0:c1], qsb[:], rot[:])
                # ---- evict latent shard + k rope ----
                dka = ring.tile([128, TB], bf16, tag="dka", name=f"dka{g}")
                nc.scalar.copy(dka[:], dkp[:])
                ckv_my = dram.tile([LSH, TB], bf16, tag="ckv_my", name=f"ckvmy{g}")
                nc.sync.dma_start(ckv_my[:], dka[0:LSH, :])
                gath = dram.tile([CD, TB], bf16, tag="gath", addr_space="Shared",
                                 name=f"gath{g}")
                nc.gpsimd.collective_compute(
                    "AllGather", mybir.AluOpType.bypass, replica_groups=rg,
                    ins=[ckv_my.opt()], outs=[gath.opt()])
                gaths.append(gath)
                # rope rows live at partitions 64:128
                rotk = ring.tile([128, TB], bf16, tag="rotk", name=f"rotk{g}")
                nc.sync.dma_start(rotk[64:96, :], dka[96:128, :])
                nc.sync.dma_start(rotk[96:128, :], dka[64:96, :])
                ktm = ring.tile([128, TB], bf16, tag="ktm", name=f"ktm{g}")
                nc.vector.tensor_mul(ktm[64:128, :], dka[64:128, :],
                                     kcos_sb[64:128, p0:p1])
                nc.vector.tensor_mul(rotk[64:128, :], rotk[64:128, :],
                                     ksin_sb[64:128, p0:p1])
                nc.vector.tensor_add(ktm[64:128, :], ktm[64:128, :],
                                     rotk[64:128, :])
                nc.sync.dma_start(kT_sb[0:32, c0:c1], ktm[64:96, :])
                nc.sync.dma_start(kT_sb[64:96, c0:c1], ktm[96:128, :])

            # deferred BCD weight loads (complete during remaining A blocks)
            nc.sync.dma_start(wupk_sb[:], wupk.rearrange("(t p) w -> p t w", p=128))
            nc.sync.dma_start(wupv_sb[:], wupv.rearrange("(t p) w -> p t w", p=128))
            nc.sync.dma_start(masks_sb[:], masks2)
            nc.sync.dma_start(wo_sb[:], wo.rearrange("(h p) w -> p h w", p=128))

            # ================ phases B/C/D per block ================
            for g in range(NG):
                c0, c1 = g * TB, (g + 1) * TB
                b, qb = g // (S // TB), g % (S // TB)
                off = b * S
                # ---- B: k_nope + v from gathered latent ----
                cb = ring.tile([128, LT, TB], bf16, tag="cb", name=f"cb{g}")
                nc.sync.dma_start(cb[:], gaths[g].rearrange("(l p) w -> p l w", p=128))
                kcp = psp.tile([KRR, TB], f32, tag="mid", bufs=2, name=f"kcp{g}")
                for lt in range(LT):
                    nc.tensor.matmul(kcp[:], wupk_sb[:, lt, :], cb[:, lt, :],
                                     start=(lt == 0), stop=(lt == LT - 1))
                kcs = ring.tile([KRR, TB], bf16, tag="kcs", name=f"kcs{g}")
                nc.scalar.copy(kcs[:], kcp[:])
                nc.sync.dma_start(kT_sb[32:64, c0:c1], kcs[0:32, :])
                nc.sync.dma_start(kT_sb[96:128, c0:c1], kcs[32:64, :])
                for tt in range(TB // 128):
                    vp = psp.tile([128, HEAD_DIM], f32, tag="mid", bufs=2,
                                  name=f"vp{g}_{tt}")
                    for lt in range(LT):
                        nc.tensor.matmul(vp[:],
                                         cb[:, lt, tt * 128:(tt + 1) * 128],
                                         wupv_sb[:, lt, :],
                                         start=(lt == 0), stop=(lt == LT - 1))
                    nc.scalar.copy(v_sb[:, g * (TB // 128) + tt, :], vp[:])

                # ---- C: attention for q-block (b, qb), 4 heads ----
                atn = ring.tile([128, HPC, TB], bf16, tag="atn", name=f"atn{g}")
                nkt = (qb + 1) * NJ
                npair = nkt // 2
                for h in range(HPC):
                    qsl = qT_sb[:, h, off + qb * TB: off + (qb + 1) * TB]
                    ops = psp.tile([128, TB], f32, tag="mid", bufs=2,
                                   name=f"ops{g}_{h}")
                    acc = ring.tile([128, 2 * TB], F32R, tag="acc", bufs=2,
                                    name=f"acc{g}_{h}")
                    pts = [None, None, None]
                    # depth-2 software pipeline: scores run 2 pairs ahead
                    # of the AV accumulation.
                    # q0s[kt] = first causally-active q column of k-tile kt;
                    # only the last diagonal pair is restricted (the first
                    # one is nearly full anyway).
                    def q0_of(kt):
                        j = kt - qb * NJ
                        return j * 128 if j >= 2 else 0

                    for p in range(npair + 2):
                        if p < npair:
                            sc2 = psp.tile([128, 2 * TB], f32, tag="big", bufs=2,
                                           name=f"sc2_{g}_{h}_{p}")
                            for u in range(2):
                                kt = 2 * p + u
                                q0 = q0_of(kt)
                                nc.tensor.matmul(
                                    sc2[:, u * TB + q0:(u + 1) * TB],
                                    kT_sb[:, off + kt * 128: off + (kt + 1) * 128],
                                    qsl[:, q0:],
                                    start=True, stop=True)
                            pt2 = ring.tile([128, 2 * TB], bf16, tag="pt2", bufs=3,
                                            name=f"pt2_{g}_{h}_{p}")
                            if q0_of(2 * p) > 0:
                                for u in range(2):
                                    q0 = q0_of(2 * p + u)
                                    nc.scalar.activation(
                                        pt2[:, u * TB + q0:(u + 1) * TB],
                                        sc2[:, u * TB + q0:(u + 1) * TB], EXP)
                            else:
                                nc.scalar.activation(pt2[:], sc2[:], EXP)
                            jp = p - qb * (NJ // 2)
                            if jp >= 0:
                                # full-width: also zeroes stale data left of q0
                                nc.vector.tensor_mul(pt2[:], pt2[:],
                                                     masks_sb[:, jp, :])
                            pts[p % 3] = pt2
                        pp = p - 2
                        if pp >= 0:
                            ppt = pts[pp % 3]
                            for u in range(2):
                                kt = 2 * pp + u
                                q0 = q0_of(kt)
                                nc.tensor.matmul(
                                    ops[:, q0:],
                                    v_sb[:, b * (S // 128) + kt, :],
                                    ppt[:, u * TB + q0:(u + 1) * TB],
                                    start=(kt == 0), stop=(kt == nkt - 1))
                            # denominator partial sums on DVE, emitted after
                            # the AV matmuls so they don't delay the next
                            # pair's mask-mul in the DVE stream
                            if pp == 0:
                                nc.vector.tensor_copy(acc[:], ppt[:])
                            else:
                                nc.vector.tensor_add(acc[:], acc[:], ppt[:])
                    # denominators: reduce acc halves with a ones-matmul
                    sps = psp.tile([1, TB], f32, tag="sum", bufs=2,
                                   name=f"sps{g}_{h}")
                    for u in range(2):
                        nc.tensor.matmul(sps[:], ones32_sb[:],
                                         acc[:, u * TB:(u + 1) * TB],
                                         start=(u == 0), stop=(u == 1))
                    rec = ring.tile([1, TB], f32, tag="rec", bufs=1,
                                    name=f"rec{g}_{h}")
                    nc.vector.reciprocal_approx_fast(rec[:], sps[:])
                    rbs = ring.tile([128, TB], f32, tag="rbs", name=f"rbs{g}_{h}")
                    nc.gpsimd.partition_broadcast(rbs[:], rec[:])
                    nc.vector.tensor_mul(atn[:, h, :], ops[:], rbs[:])

                # ---- D: partial o_proj (1024-wide moving) ----
                for T in range(TB // 128):
                    for n in range(HIDDEN // 1024):
                        ps2 = psp.tile([128, 2 * TB], f32, tag="big", bufs=2,
                                       name=f"od_{g}_{T}_{n}")
                        for half in range(2):
                            for h2 in range(HPC):
                                nc.tensor.matmul(
                                    ps2[:, half * TB:(half + 1) * TB],
                                    atn[:, h2, T * 128:(T + 1) * 128],
                                    wo_sb[:, h2, n * 1024 + half * TB:
                                          n * 1024 + (half + 1) * TB],
                                    start=(h2 == 0), stop=(h2 == HPC - 1))
                        osb = ring.tile([128, 2 * TB], bf16, tag="osb", bufs=2,
                                        name=f"osb{g}_{T}_{n}")
                        if n % 2 == 0:
                            nc.scalar.copy(osb[:], ps2[:])
                        else:
                            nc.vector.tensor_copy(osb[:], ps2[:])
                        nc.sync.dma_start(
                            outp[c0 + T * 128: c0 + (T + 1) * 128,
                                 n * 1024:(n + 1) * 1024], osb[:])

    nc.compile()
    return nc


def make_in_maps(hidden_states, Wq, Wkr, Wdk, Wupk, Wupv, Wo):
    """Host-side sharding + layout prep (bf16). Returns per-core input dicts."""
    import ml_dtypes
    bf16 = ml_dtypes.bfloat16
    scale = 1.0 / np.sqrt(np.float32(HEAD_DIM))

    hidT = np.ascontiguousarray(
        np.asarray(hidden_states, np.float32).reshape(NT, HIDDEN).T).astype(bf16)

    cos_t, sin_t = _rope_tables(S)                     # [128, S] f32
    qcos = (cos_t * scale).astype(bf16)
    qsin = (np.concatenate([-sin_t[0:64], sin_t[64:128]], axis=0) * scale).astype(bf16)
    kcos = np.zeros((128, S), np.float32)
    ksin = np.zeros((128, S), np.float32)
    kcos[64:96] = cos_t[0:32]
    kcos[96:128] = cos_t[64:96]
    ksin[64:96] = -sin_t[0:32]
    ksin[96:128] = sin_t[64:96]
    kcos = kcos.astype(bf16)
    ksin = ksin.astype(bf16)

    k_idx = np.arange(128)[:, None]
    q_idx = np.arange(TB)[None, :]
    m1 = np.stack(
        [(q_idx >= j * 128 + k_idx).astype(np.float32) for j in range(NJ)],
        axis=1)                                        # [128, NJ, TB]
    masks2 = np.concatenate(
        [np.stack([m1[:, 0], m1[:, 2]], axis=1),
         np.stack([m1[:, 1], m1[:, 3]], axis=1)], axis=2).astype(bf16)
    # masks2[:, i, 0:TB] = mask_{2i}, masks2[:, i, TB:2TB] = mask_{2i+1}

    Wq = np.asarray(Wq, np.float32)
    Wkr = np.asarray(Wkr, np.float32)
    Wdk = np.asarray(Wdk, np.float32)
    Wupk = np.asarray(Wupk, np.float32)
    Wupv = np.asarray(Wupv, np.float32)
    Wo = np.asarray(Wo, np.float32)

    in_maps = []
    for c in range(NCORES):
        wq_t = np.ascontiguousarray(Wq[QR * c:QR * (c + 1)].T).astype(bf16)
        wdkkr_t = np.ascontiguousarray(
            np.concatenate([Wdk[LSH * c:LSH * (c + 1)],
                            Wkr[KRR * c:KRR * (c + 1)]], axis=0).T).astype(bf16)
        wupk_t = np.ascontiguousarray(Wupk[KRR * c:KRR * (c + 1)].T).astype(bf16)
        wupv_t = np.ascontiguousarray(
            Wupv[HEAD_DIM * c:HEAD_DIM * (c + 1)].T).astype(bf16)
        wo_t = np.ascontiguousarray(Wo[:, QR * c:QR * (c + 1)].T).astype(bf16)
        in_maps.append({
            "hidT": hidT, "wq_t": wq_t, "wdkkr_t": wdkkr_t,
            "wupk_t": wupk_t, "wupv_t": wupv_t, "wo_t": wo_t,
            "qcos": qcos, "qsin": qsin, "kcos": kcos, "ksin": ksin,
            "masks2": masks2,
        })
    return in_maps


_NC_CACHE = {}


def _get_program(key=0):
    if key not in _NC_CACHE:
        _NC_CACHE[key] = build_program()
    return _NC_CACHE[key]


def finish_output(res):
    out = res.results[0]["out_part"].astype(np.float32)
    for i in range(1, NCORES):
        out = out + res.results[i]["out_part"].astype(np.float32)
    return out.reshape(B, S, HIDDEN).astype(np.float32)


def kernel(hidden_states, Wq, Wkr, Wdk, Wupk, Wupv, Wo):
    from concourse.bass_utils import run_bass_kernel_spmd

    in_maps = make_in_maps(hidden_states, Wq, Wkr, Wdk, Wupk, Wupv, Wo)
    nc = _get_program()
    res = run_bass_kernel_spmd(nc, in_maps, list(range(NCORES)))
    return finish_output(res)
